# revision 14
# baseline (speedup 1.0000x reference)
"""Trainium2 Bass kernel for nn_Attention_29807073034381.

Multi-head attention (B=2, S=2048, E=1024, H=16, D=64) with LoRA-augmented QKV
projection, sharded 2-heads-per-core across 8 NeuronCores (tensor parallel).

Per-core plan (heads hA=2c, hB=2c+1), all compute in bf16 with fp32 PSUM:
  - host passes x transposed (xT [E, T]) so every matmul contraction sits on
    the SBUF partition axis; weights pre-transposed/sliced per core; softmax
    scaling folded into Wq/bias_q/B_q(lora) on the host.
  - projection: QT/KT [128, T] (head-dim on partitions), V [tok, dv] tiles
    augmented with a ones column; LoRA delta and bias ride the same PSUM
    accumulation group (xa = A @ xT computed once, augmented with a ones row).
  - attention per (b, si-chunk of 512): scoresT tiles [sj=128, si=512] for
    both heads packed into one [128, 1024] PSUM tile via K=64 row-packed
    matmuls (tile_position (0,0)/(64,0)); one ACT exp per sj-tile
    (PSUM f32 -> SBUF bf16); PV matmuls against ones-augmented V accumulate
    attnoutT and the softmax denominator in one group.
  - normalize via reciprocal_approx_fast + gpsimd partition_broadcast,
    multiply while copying PSUM->SBUF (bf16 attnT [hd=128, si]).
  - out_proj: partial = attnT.T @ woT slice -> [tok, E] bf16 partial output.
  - host sums the 8 partials in fp32 and adds out_proj_bias.

attn_mask is all-zeros in this problem's setup_inputs; a masked variant of the
graph (extra maskT input added to scores pre-exp) is built only if a nonzero
mask ever shows up.
"""

import numpy as np
import ml_dtypes
from contextlib import ExitStack

import concourse.bass as bass
import concourse.bacc as bacc
import concourse.tile as tile
import concourse.mybir as mybir
from concourse.bass_utils import run_bass_kernel_spmd
from concourse.bass import ts, ds

BF16 = mybir.dt.bfloat16
F32 = mybir.dt.float32

P = 128
E = 1024
H = 16
D = 64
B = 2
S = 2048
T = B * S            # 4096 tokens
R = 8                # lora rank
ET = E // P          # 8 e-tiles
TCH = 512            # projection token chunk
NTCH = T // TCH      # 8
SC = 512             # attention si chunk
NSC = S // SC        # 4 per batch
SJT = S // P         # 16 sj tiles per batch
NCORES = 8
SCALE = float(D) ** -0.5

_nc_cache = {}


def _build_nc(use_mask: bool):
    nc = bacc.Bacc("TRN2", target_bir_lowering=False, debug=False,
                   num_devices=NCORES)
    xT_d = nc.dram_tensor("xT", [E, T], BF16, kind="ExternalInput").ap()
    wq_d = nc.dram_tensor("wqT", [E, P], BF16, kind="ExternalInput").ap()
    wk_d = nc.dram_tensor("wkT", [E, P], BF16, kind="ExternalInput").ap()
    wv_d = nc.dram_tensor("wvT", [E, P], BF16, kind="ExternalInput").ap()
    a_d = nc.dram_tensor("aT", [E, R], BF16, kind="ExternalInput").ap()
    bq_d = nc.dram_tensor("bqT", [R, P], BF16, kind="ExternalInput").ap()
    bk_d = nc.dram_tensor("bkT", [R, P], BF16, kind="ExternalInput").ap()
    bv_d = nc.dram_tensor("bvT", [R, P], BF16, kind="ExternalInput").ap()
    qb_d = nc.dram_tensor("qb", [P, 1], F32, kind="ExternalInput").ap()
    kb_d = nc.dram_tensor("kb", [P, 1], F32, kind="ExternalInput").ap()
    vb_d = nc.dram_tensor("vb", [P, 1], F32, kind="ExternalInput").ap()
    wo_d = nc.dram_tensor("woT", [P, E], BF16, kind="ExternalInput").ap()
    mask_d = None
    if use_mask:
        mask_d = nc.dram_tensor("maskT", [B, S, S], BF16,
                                kind="ExternalInput").ap()
    out_d = nc.dram_tensor("out", [T, E], BF16, kind="ExternalOutput").ap()

    with tile.TileContext(nc) as tc, ExitStack() as ctx:
        persist = ctx.enter_context(tc.tile_pool(name="persist", bufs=1))
        work = ctx.enter_context(tc.tile_pool(name="work", bufs=2))
        expp = ctx.enter_context(tc.tile_pool(name="expp", bufs=4))
        psum = ctx.enter_context(tc.tile_pool(name="psum", bufs=2, space="PSUM"))

        # ---- persistent SBUF tensors ----
        xT = persist.tile([P, ET, T], BF16, name="xT_sb", tag="xT_sb")
        wq = persist.tile([P, ET, P], BF16, name="wq_sb", tag="wq_sb")
        wk = persist.tile([P, ET, P], BF16, name="wk_sb", tag="wk_sb")
        wv = persist.tile([P, ET, P], BF16, name="wv_sb", tag="wv_sb")
        at = persist.tile([P, ET, R], BF16, name="at_sb", tag="at_sb")
        bq = persist.tile([R, P], BF16, name="bq_sb", tag="bq_sb")
        bk = persist.tile([R, P], BF16, name="bk_sb", tag="bk_sb")
        bv = persist.tile([R, P], BF16, name="bv_sb", tag="bv_sb")
        qb = persist.tile([P, 1], F32, name="qb_sb", tag="qb_sb")
        kb = persist.tile([P, 1], F32, name="kb_sb", tag="kb_sb")
        vb = persist.tile([P, 1], F32, name="vb_sb", tag="vb_sb")
        wo = persist.tile([P, E], BF16, name="wo_sb", tag="wo_sb")
        QT = persist.tile([P, T], BF16, name="QT_sb", tag="QT_sb")
        KT = persist.tile([P, T], BF16, name="KT_sb", tag="KT_sb")
        V = persist.tile([P, T // P, 2 * (D + 1)], BF16, name="V_sb", tag="V_sb")
        xa = persist.tile([R, NTCH, TCH], BF16, name="xa_sb", tag="xa_sb")

        nc.sync.dma_start(wq[:], wq_d.rearrange("(o p) m -> p o m", p=P))
        nc.sync.dma_start(wk[:], wk_d.rearrange("(o p) m -> p o m", p=P))
        nc.sync.dma_start(wv[:], wv_d.rearrange("(o p) m -> p o m", p=P))
        nc.sync.dma_start(at[:], a_d.rearrange("(o p) m -> p o m", p=P))
        nc.sync.dma_start(bq[:], bq_d)
        nc.sync.dma_start(bk[:], bk_d)
        nc.sync.dma_start(bv[:], bv_d)
        nc.sync.dma_start(qb[:], qb_d)
        nc.sync.dma_start(kb[:], kb_d)
        nc.sync.dma_start(vb[:], vb_d)
        nc.sync.dma_start(wo[:], wo_d)
        xT_r = xT_d.rearrange("(o p) t -> p o t", p=P)
        for t in range(NTCH):
            nc.sync.dma_start(xT[:, :, ts(t, TCH)], xT_r[:, :, ts(t, TCH)])

        # ones columns for the softmax-denominator augmentation of V
        nc.vector.memset(V[:, :, D:D + 1], 1.0)
        nc.vector.memset(V[:, :, 2 * D + 1:2 * D + 2], 1.0)

        # ---- stage A pieces ----
        def proj_xa(t):
            ps = psum.tile([R, TCH], F32, name=f"xa_ps_{t}", tag="a", bufs=1)
            for e in range(ET):
                nc.tensor.matmul(ps[:], at[:, e, :], xT[:, e, ts(t, TCH)],
                                 start=(e == 0), stop=(e == ET - 1))
            nc.vector.tensor_copy(out=xa[:, t, :], in_=ps[:])

        def proj_qk(t, is_q):
            w, bb, bcol, dst = (wq, bq, qb, QT) if is_q else (wk, bk, kb, KT)
            nm = "q" if is_q else "k"
            ps = psum.tile([P, TCH], F32, name=f"{nm}_ps_{t}", tag="a", bufs=1)
            for e in range(ET):
                nc.tensor.matmul(ps[:], w[:, e, :], xT[:, e, ts(t, TCH)],
                                 start=(e == 0), stop=False)
            nc.tensor.matmul(ps[:], bb[:], xa[:, t, :], start=False, stop=True)
            nc.vector.tensor_scalar_add(dst[:, ts(t, TCH)], ps[:], bcol[:])

        def proj_v(t):
            for s4 in range(TCH // P):
                jt = t * (TCH // P) + s4
                ps = psum.tile([P, P], F32, name=f"v_ps_{jt}", tag="a", bufs=1)
                for e in range(ET):
                    nc.tensor.matmul(ps[:], xT[:, e, ds(t * TCH + s4 * P, P)],
                                     wv[:, e, :], start=(e == 0), stop=False)
                nc.tensor.matmul(ps[:], xa[:, t, ds(s4 * P, P)], bv[:],
                                 start=False, stop=True)
                nc.vector.tensor_copy(
                    out=V[:, jt].rearrange("p (g c) -> p g c", g=2)[:, :, 0:D],
                    in_=ps.rearrange("p (g c) -> p g c", g=2))

        # ---- stage B: attention sj-loop for one (b, si-chunk) ----
        def attn_compute(b, sci):
            si0 = b * S + sci * SC
            pvA = psum.tile([D + 1, SC], F32, name=f"pvA_{b}_{sci}", tag="pv",
                            bufs=3)
            pvB = psum.tile([D + 1, SC], F32, name=f"pvB_{b}_{sci}", tag="pv",
                            bufs=3)
            for sjt in range(SJT):
                jt = b * SJT + sjt
                scs = psum.tile([P, 2 * SC], F32, name=f"scs_{b}_{sci}_{sjt}",
                                tag="sc")
                nc.tensor.matmul(scs[:, 0:SC], KT[0:D, ds(jt * P, P)],
                                 QT[0:D, ds(si0, SC)], start=True, stop=True,
                                 tile_position=(0, 0))
                nc.tensor.matmul(scs[:, SC:2 * SC], KT[D:P, ds(jt * P, P)],
                                 QT[D:P, ds(si0, SC)], start=True, stop=True,
                                 tile_position=(64, 0))
                if use_mask:
                    mt = work.tile([P, SC], BF16, name=f"mt_{b}_{sci}_{sjt}",
                                   tag="mask", bufs=3)
                    nc.sync.dma_start(
                        mt[:], mask_d[b, ds(jt % SJT * P, P), ds(sci * SC, SC)])
                    nc.vector.tensor_tensor(
                        out=scs.rearrange("p (g c) -> p g c", g=2),
                        in0=scs.rearrange("p (g c) -> p g c", g=2),
                        in1=mt[:, None, :].to_broadcast([P, 2, SC]),
                        op=mybir.AluOpType.add)
                expab = expp.tile([P, 2 * SC], BF16, name=f"ex_{b}_{sci}_{sjt}",
                                  tag="exp")
                nc.scalar.activation(expab[:], scs[:],
                                     mybir.ActivationFunctionType.Exp)
                nc.tensor.matmul(pvA[:], V[:, jt, 0:D + 1], expab[:, 0:SC],
                                 start=(sjt == 0), stop=(sjt == SJT - 1))
                nc.tensor.matmul(pvB[:], V[:, jt, D + 1:2 * (D + 1)],
                                 expab[:, SC:2 * SC],
                                 start=(sjt == 0), stop=(sjt == SJT - 1))
            return pvA, pvB

        # ---- normalize + out_proj for a finished (b, si-chunk) ----
        def attn_finish(b, sci, pvA, pvB):
            si0 = b * S + sci * SC
            # normalize -> attnT [hd 128, si]
            attnT = work.tile([P, SC], BF16, name=f"attnT_{b}_{sci}", tag="attnT")
            for hh, pv in ((0, pvA), (1, pvB)):
                den = work.tile([1, SC], F32, name=f"den_{b}_{sci}_{hh}",
                                tag="den", bufs=4)
                nc.vector.tensor_copy(out=den[:], in_=pv[D:D + 1, :])
                rec = work.tile([1, SC], F32, name=f"rec_{b}_{sci}_{hh}",
                                tag="rec", bufs=4)
                nc.vector.reciprocal_approx_fast(out=rec[:], in_=den[:])
                bc = work.tile([D, SC], F32, name=f"bc_{b}_{sci}_{hh}",
                               tag="bc", bufs=4)
                nc.gpsimd.partition_broadcast(bc[:], rec[:])
                nc.vector.tensor_tensor(out=attnT[hh * D:(hh + 1) * D, :],
                                        in0=pv[0:D, :], in1=bc[:],
                                        op=mybir.AluOpType.mult)
                # V-bias: P@(V + 1⊗vb) = PV + denom⊗vb -> add vb post-normalize
                nc.vector.tensor_scalar_add(attnT[hh * D:(hh + 1) * D, :],
                                            attnT[hh * D:(hh + 1) * D, :],
                                            vb[hh * D:(hh + 1) * D, :])
            # out_proj for this chunk
            for tt in range(SC // P):
                tok0 = si0 + tt * P
                ops = psum.tile([P, E], F32, name=f"o_ps_{b}_{sci}_{tt}",
                                tag="sc")
                for ne in range(E // 512):
                    nc.tensor.matmul(ops[:, ts(ne, 512)], attnT[:, ts(tt, P)],
                                     wo[:, ts(ne, 512)], start=True, stop=True)
                outt = work.tile([P, E], BF16, name=f"outt_{b}_{sci}_{tt}",
                                 tag="outt", bufs=3)
                nc.vector.tensor_copy(out=outt[:], in_=ops[:])
                nc.sync.dma_start(out_d[ds(tok0, P), :], outt[:])

        # ---- emission order: prologue KV(b0) + Q0, then a chunk-level
        # software pipeline: sj-loop(i) first, THEN finish(i-1) (normalize on
        # DVE/GpSimd overlaps sj-loop(i); out_proj matmuls slot in behind its
        # scores so the PE never sees a >3.4us gap -> HAM stays at 8/8),
        # with remaining projection chunks woven in for the same reason.
        for t in range(4):
            proj_xa(t)
            proj_qk(t, False)
            proj_v(t)
        proj_qk(0, True)
        chunks = [(0, s) for s in range(NSC)] + [(1, s) for s in range(NSC)]
        prev = None
        for i, (b, sci) in enumerate(chunks):
            pv = attn_compute(b, sci)
            if prev is not None:
                attn_finish(*prev)
            prev = (b, sci, *pv)
            if i < 4:                   # weave b1 KV + next Q chunks
                t2 = 4 + i
                proj_xa(t2)
                proj_qk(t2, False)
                proj_v(t2)
                proj_qk(i + 1, True)
            elif i + 1 < NTCH:          # remaining Q chunks (5..7)
                proj_qk(i + 1, True)
        attn_finish(*prev)

    nc.compile()
    return nc


def _get_nc(use_mask: bool):
    if use_mask not in _nc_cache:
        _nc_cache[use_mask] = _build_nc(use_mask)
    return _nc_cache[use_mask]


def _prep_in_maps(x, attn_mask, in_proj_weight, in_proj_bias, out_proj_weight,
                  lora_a, lora_b, use_mask):
    bf = ml_dtypes.bfloat16
    xT = np.ascontiguousarray(x.reshape(T, E).T).astype(bf)
    aT = np.ascontiguousarray(lora_a.T).astype(bf)
    maskT = None
    if use_mask:
        maskT = np.ascontiguousarray(attn_mask.transpose(0, 2, 1)).astype(bf)
    in_maps = []
    for c in range(NCORES):
        h0 = 2 * c
        qs = slice(h0 * D, (h0 + 2) * D)
        ks = slice(E + h0 * D, E + (h0 + 2) * D)
        vs = slice(2 * E + h0 * D, 2 * E + (h0 + 2) * D)
        m = {
            "xT": xT,
            "aT": aT,
            "wqT": np.ascontiguousarray((in_proj_weight[qs, :] * SCALE).T).astype(bf),
            "wkT": np.ascontiguousarray(in_proj_weight[ks, :].T).astype(bf),
            "wvT": np.ascontiguousarray(in_proj_weight[vs, :].T).astype(bf),
            "bqT": np.ascontiguousarray(lora_b[qs, :].T * SCALE).astype(bf),
            "bkT": np.ascontiguousarray(lora_b[ks, :].T).astype(bf),
            "bvT": np.ascontiguousarray(lora_b[vs, :].T).astype(bf),
            "qb": np.ascontiguousarray((in_proj_bias[qs] * SCALE)[:, None]).astype(np.float32),
            "kb": np.ascontiguousarray(in_proj_bias[ks][:, None]).astype(np.float32),
            "vb": np.ascontiguousarray(in_proj_bias[vs][:, None]).astype(np.float32),
            "woT": np.ascontiguousarray(out_proj_weight[:, h0 * D:(h0 + 2) * D].T).astype(bf),
        }
        if use_mask:
            m["maskT"] = maskT
        in_maps.append(m)
    return in_maps


def kernel(x, attn_mask, in_proj_weight, in_proj_bias, out_proj_weight,
           out_proj_bias, lora_a, lora_b, _trace=False):
    x = np.asarray(x, dtype=np.float32)
    attn_mask = np.asarray(attn_mask, dtype=np.float32)
    in_proj_weight = np.asarray(in_proj_weight, dtype=np.float32)
    in_proj_bias = np.asarray(in_proj_bias, dtype=np.float32)
    out_proj_weight = np.asarray(out_proj_weight, dtype=np.float32)
    out_proj_bias = np.asarray(out_proj_bias, dtype=np.float32)
    lora_a = np.asarray(lora_a, dtype=np.float32)
    lora_b = np.asarray(lora_b, dtype=np.float32)

    use_mask = bool(np.any(attn_mask))
    nc = _get_nc(use_mask)
    in_maps = _prep_in_maps(x, attn_mask, in_proj_weight, in_proj_bias,
                            out_proj_weight, lora_a, lora_b, use_mask)
    res = run_bass_kernel_spmd(nc, in_maps, core_ids=list(range(NCORES)),
                               trace=_trace)
    acc = np.zeros((T, E), np.float32)
    for c in range(NCORES):
        acc += res.results[c]["out"].astype(np.float32)
    acc += out_proj_bias[None, :]
    out = acc.reshape(B, S, E)
    if _trace:
        kernel._last_exec_time_ns = res.exec_time_ns
        kernel._last_trace = (res.instructions_and_trace[1]
                              if res.instructions_and_trace else None)
    return out


# revision 24
# speedup vs baseline: 1.2145x; 1.2145x over previous
"""Trainium2 Bass kernel for nn_Attention_29807073034381.

Multi-head attention (B=2, S=2048, E=1024, H=16, D=64) with LoRA-augmented QKV
projection, sharded 2-heads-per-core across 8 NeuronCores (tensor parallel).

Per-core plan (heads hA=2c, hB=2c+1), all compute in bf16 with fp32 PSUM:
  - host passes x pre-transposed and pre-tiled ([128, tchunk, etile, 512]
    contiguous per chunk, so every DMA line is 8KB) so matmul contractions sit
    on the SBUF partition axis; weights pre-transposed/sliced per core; the
    softmax scaling is folded into Wq/bias_q/B_q(lora) on the host.
  - projection: QT/KT [128, T] (head-dim on partitions, weight-stationary);
    V computed as VT then PE-transposed into [tok, dv] tiles augmented with a
    ones column; LoRA delta rides the same PSUM accumulation group (xa = A @
    xT computed once); Q/K bias added as a per-partition scalar at copy time.
  - attention per (b, si-chunk of 512): scoresT tiles [sj=128, si=512] for
    both heads packed into one [128, 1024] PSUM tile via K=64 row-packed
    matmuls (tile_position (0,0)/(64,0)); one ACT exp per sj-tile
    (PSUM f32 -> SBUF bf16); PV matmuls against ones-augmented V accumulate
    attnoutT and the softmax denominator in one group.
  - normalize via reciprocal_approx_fast + gpsimd partition_broadcast,
    multiply while copying PSUM->SBUF (bf16 attnT [hd=128, si]); V-bias is
    added post-normalize (P@(V + 1*vb) = PV + denom*vb).
  - out_proj: partial = attnT.T @ woT slice -> [tok, E] bf16 partial output.
  - host sums the 8 partials in fp32 and adds out_proj_bias.

Scheduling: emission order is a chunk-level software pipeline. All projection
work is chopped into ~2-4us "morsels" on a queue; the attention sj-loop pops
one morsel per sj iteration so the ScalarE (exp) never starves while the PE
fills its slack with projection work, and finish(i-1) (normalize + out_proj)
is emitted after compute(i) so the PE never idles long enough to re-throttle
(HAM).

attn_mask is all-zeros in this problem's setup_inputs; a masked variant of
the graph (maskT added to scores pre-exp) is built only if a nonzero mask
ever shows up.
"""

import numpy as np
import ml_dtypes
from collections import deque
from contextlib import ExitStack

import concourse.bass as bass
import concourse.bacc as bacc
import concourse.tile as tile
import concourse.mybir as mybir
from concourse.bass_utils import run_bass_kernel_spmd
from concourse.bass import ts, ds
from concourse.masks import make_identity

BF16 = mybir.dt.bfloat16
F32 = mybir.dt.float32

P = 128
E = 1024
H = 16
D = 64
B = 2
S = 2048
T = B * S            # 4096 tokens
R = 8                # lora rank
ET = E // P          # 8 e-tiles
TCH = 512            # projection token chunk
NTCH = T // TCH      # 8
SC = 512             # attention si chunk
NSC = S // SC        # 4 per batch
SJT = S // P         # 16 sj tiles per batch
NCORES = 8
SCALE = float(D) ** -0.5
POP_IN_LOOP = True
DEBUG_DUMP = False

_nc_cache = {}


def _build_nc(use_mask: bool):
    nc = bacc.Bacc("TRN2", target_bir_lowering=False, debug=False,
                   num_devices=NCORES)
    xT_d = nc.dram_tensor("xT", [P, NTCH, ET, TCH], BF16,
                          kind="ExternalInput").ap()
    wq_d = nc.dram_tensor("wqT", [P, ET, P], BF16, kind="ExternalInput").ap()
    wk_d = nc.dram_tensor("wkT", [P, ET, P], BF16, kind="ExternalInput").ap()
    wv_d = nc.dram_tensor("wvT", [P, ET, P], BF16, kind="ExternalInput").ap()
    a_d = nc.dram_tensor("aT", [P, ET, R], BF16, kind="ExternalInput").ap()
    bq_d = nc.dram_tensor("bqT", [R, P], BF16, kind="ExternalInput").ap()
    bk_d = nc.dram_tensor("bkT", [R, P], BF16, kind="ExternalInput").ap()
    bv_d = nc.dram_tensor("bvT", [R, P], BF16, kind="ExternalInput").ap()
    qb_d = nc.dram_tensor("qb", [P, 1], F32, kind="ExternalInput").ap()
    kb_d = nc.dram_tensor("kb", [P, 1], F32, kind="ExternalInput").ap()
    vb_d = nc.dram_tensor("vb", [P, 1], F32, kind="ExternalInput").ap()
    wo_d = nc.dram_tensor("woT", [P, E], BF16, kind="ExternalInput").ap()
    mask_d = None
    if use_mask:
        mask_d = nc.dram_tensor("maskT", [B, S, S], BF16,
                                kind="ExternalInput").ap()
    out_d = nc.dram_tensor("out", [T, E], BF16, kind="ExternalOutput").ap()
    if DEBUG_DUMP:
        dqt_d = nc.dram_tensor("dbg_qt", [P, T], BF16, kind="ExternalOutput").ap()
        dkt_d = nc.dram_tensor("dbg_kt", [P, T], BF16, kind="ExternalOutput").ap()
        dv_d = nc.dram_tensor("dbg_v", [P, (T // P) * 2 * (D + 1)], BF16,
                              kind="ExternalOutput").ap()
        dxa_d = nc.dram_tensor("dbg_xa", [R, T], BF16, kind="ExternalOutput").ap()

    with tile.TileContext(nc) as tc, ExitStack() as ctx:
        persist = ctx.enter_context(tc.tile_pool(name="persist", bufs=1))
        work = ctx.enter_context(tc.tile_pool(name="work", bufs=2))
        expp = ctx.enter_context(tc.tile_pool(name="expp", bufs=6))
        psum = ctx.enter_context(tc.tile_pool(name="psum", bufs=2, space="PSUM"))

        # ---- persistent SBUF tensors ----
        xT = persist.tile([P, NTCH, ET, TCH], BF16, name="xT_sb", tag="xT_sb")
        wq = persist.tile([P, ET, P], BF16, name="wq_sb", tag="wq_sb")
        wk = persist.tile([P, ET, P], BF16, name="wk_sb", tag="wk_sb")
        wv = persist.tile([P, ET, P], BF16, name="wv_sb", tag="wv_sb")
        at = persist.tile([P, ET, R], BF16, name="at_sb", tag="at_sb")
        bq = persist.tile([R, P], BF16, name="bq_sb", tag="bq_sb")
        bk = persist.tile([R, P], BF16, name="bk_sb", tag="bk_sb")
        bv = persist.tile([R, P], BF16, name="bv_sb", tag="bv_sb")
        qb = persist.tile([P, 1], F32, name="qb_sb", tag="qb_sb")
        kb = persist.tile([P, 1], F32, name="kb_sb", tag="kb_sb")
        vb = persist.tile([P, 1], F32, name="vb_sb", tag="vb_sb")
        wo = persist.tile([P, E], BF16, name="wo_sb", tag="wo_sb")
        ident = persist.tile([P, P], BF16, name="ident_sb", tag="ident_sb")
        QT = persist.tile([P, T], BF16, name="QT_sb", tag="QT_sb")
        KT = persist.tile([P, T], BF16, name="KT_sb", tag="KT_sb")
        V = persist.tile([P, T // P, 2 * (D + 1)], BF16, name="V_sb", tag="V_sb")
        xa = persist.tile([R, NTCH, TCH], BF16, name="xa_sb", tag="xa_sb")

        nc.sync.dma_start(wq[:], wq_d)
        nc.sync.dma_start(wk[:], wk_d)
        nc.sync.dma_start(wv[:], wv_d)
        nc.sync.dma_start(at[:], a_d)
        nc.sync.dma_start(bq[:], bq_d)
        nc.sync.dma_start(bk[:], bk_d)
        nc.sync.dma_start(bv[:], bv_d)
        nc.sync.dma_start(qb[:], qb_d)
        nc.sync.dma_start(kb[:], kb_d)
        nc.sync.dma_start(vb[:], vb_d)
        nc.sync.dma_start(wo[:], wo_d)
        for t in range(NTCH):
            nc.sync.dma_start(xT[:, t], xT_d[:, t])

        make_identity(nc, ident[:])
        # ones columns for the softmax-denominator augmentation of V
        nc.vector.memset(V[:, :, D:D + 1], 1.0)
        nc.vector.memset(V[:, :, 2 * D + 1:2 * D + 2], 1.0)

        # ---- stage A morsels ----
        def proj_xa(t):
            ps = psum.tile([R, TCH], F32, name=f"xa_ps_{t}", tag="a")
            for e in range(ET):
                nc.tensor.matmul(ps[:], at[:, e, :], xT[:, t, e, :],
                                 start=(e == 0), stop=(e == ET - 1))
            nc.vector.tensor_copy(out=xa[:, t, :], in_=ps[:])

        def proj_qk(t, is_q):
            w, bb, bcol, dst = (wq, bq, qb, QT) if is_q else (wk, bk, kb, KT)
            nm = "q" if is_q else "k"
            ps = psum.tile([P, TCH], F32, name=f"{nm}_ps_{t}", tag="a")
            for e in range(ET):
                nc.tensor.matmul(ps[:], w[:, e, :], xT[:, t, e, :],
                                 start=(e == 0), stop=False)
            nc.tensor.matmul(ps[:], bb[:], xa[:, t, :], start=False, stop=True)
            nc.vector.tensor_scalar_add(dst[:, ts(t, TCH)], ps[:], bcol[:])

        vt_tiles = {}

        def proj_v_mm(t):
            ps = psum.tile([P, TCH], F32, name=f"vt_ps_{t}", tag="a")
            for e in range(ET):
                nc.tensor.matmul(ps[:], wv[:, e, :], xT[:, t, e, :],
                                 start=(e == 0), stop=False)
            nc.tensor.matmul(ps[:], bv[:], xa[:, t, :], start=False, stop=True)
            vt_sb = work.tile([P, TCH], BF16, name=f"vt_sb_{t}", tag="vt",
                              bufs=2)
            nc.vector.tensor_copy(out=vt_sb[:], in_=ps[:])
            vt_tiles[t] = vt_sb

        def proj_v_tr(t, s4s):
            vt_sb = vt_tiles[t]
            for s4 in s4s:
                jt = t * (TCH // P) + s4
                pt = psum.tile([P, P], BF16, name=f"vtr_ps_{jt}", tag="a")
                nc.tensor.transpose(pt[:], vt_sb[:, ds(s4 * P, P)], ident[:])
                nc.vector.tensor_copy(
                    out=V[:, jt].rearrange("p (g c) -> p g c", g=2)[:, :, 0:D],
                    in_=pt.rearrange("p (g c) -> p g c", g=2))

        # ---- stage B: attention sj-loop; pops one morsel per sj tile ----
        work_q = deque()

        def attn_compute(b, sci, npop=1):
            si0 = b * S + sci * SC
            pvA = psum.tile([D + 1, SC], F32, name=f"pvA_{b}_{sci}", tag="pv")
            pvB = psum.tile([D + 1, SC], F32, name=f"pvB_{b}_{sci}", tag="pv")
            for sjt in range(SJT):
                # emit queued projection morsels BEFORE this iteration so
                # their tiles are written earlier in PE program order than
                # the scores/PV matmuls that read them
                for _ in range(npop):
                    if work_q:
                        work_q.popleft()()
                jt = b * SJT + sjt
                scs = psum.tile([P, 2 * SC], F32, name=f"scs_{b}_{sci}_{sjt}",
                                tag="sc")
                nc.tensor.matmul(scs[:, 0:SC], KT[0:D, ds(jt * P, P)],
                                 QT[0:D, ds(si0, SC)], start=True, stop=True,
                                 tile_position=(0, 0))
                nc.tensor.matmul(scs[:, SC:2 * SC], KT[D:P, ds(jt * P, P)],
                                 QT[D:P, ds(si0, SC)], start=True, stop=True,
                                 tile_position=(64, 0))
                if use_mask:
                    mt = work.tile([P, SC], BF16, name=f"mt_{b}_{sci}_{sjt}",
                                   tag="mask", bufs=3)
                    nc.sync.dma_start(
                        mt[:], mask_d[b, ds(sjt * P, P), ds(sci * SC, SC)])
                    nc.vector.tensor_tensor(
                        out=scs.rearrange("p (g c) -> p g c", g=2),
                        in0=scs.rearrange("p (g c) -> p g c", g=2),
                        in1=mt[:, None, :].to_broadcast([P, 2, SC]),
                        op=mybir.AluOpType.add)
                expab = expp.tile([P, 2 * SC], BF16, name=f"ex_{b}_{sci}_{sjt}",
                                  tag="exp")
                nc.scalar.activation(expab[:], scs[:],
                                     mybir.ActivationFunctionType.Exp)
                nc.tensor.matmul(pvA[:], V[:, jt, 0:D + 1], expab[:, 0:SC],
                                 start=(sjt == 0), stop=(sjt == SJT - 1))
                nc.tensor.matmul(pvB[:], V[:, jt, D + 1:2 * (D + 1)],
                                 expab[:, SC:2 * SC],
                                 start=(sjt == 0), stop=(sjt == SJT - 1))
            return pvA, pvB

        # ---- normalize + out_proj for a finished (b, si-chunk) ----
        def attn_finish(b, sci, pvA, pvB):
            si0 = b * S + sci * SC
            attnT = work.tile([P, SC], BF16, name=f"attnT_{b}_{sci}",
                              tag="attnT")
            for hh, pv in ((0, pvA), (1, pvB)):
                den = work.tile([1, SC], F32, name=f"den_{b}_{sci}_{hh}",
                                tag="den", bufs=4)
                nc.vector.tensor_copy(out=den[:], in_=pv[D:D + 1, :])
                rec = work.tile([1, SC], F32, name=f"rec_{b}_{sci}_{hh}",
                                tag="rec", bufs=4)
                nc.vector.reciprocal_approx_fast(out=rec[:], in_=den[:])
                bc = work.tile([D, SC], F32, name=f"bc_{b}_{sci}_{hh}",
                               tag="bc", bufs=4)
                nc.gpsimd.partition_broadcast(bc[:], rec[:])
                nc.vector.tensor_tensor(out=attnT[hh * D:(hh + 1) * D, :],
                                        in0=pv[0:D, :], in1=bc[:],
                                        op=mybir.AluOpType.mult)
                # V-bias: P@(V + 1*vb) = PV + denom*vb -> add vb post-normalize
                nc.vector.tensor_scalar_add(attnT[hh * D:(hh + 1) * D, :],
                                            attnT[hh * D:(hh + 1) * D, :],
                                            vb[hh * D:(hh + 1) * D, :])
            for tt in range(SC // P):
                tok0 = si0 + tt * P
                ops = psum.tile([P, E], F32, name=f"o_ps_{b}_{sci}_{tt}",
                                tag="sc")
                for ne in range(E // 512):
                    nc.tensor.matmul(ops[:, ts(ne, 512)], attnT[:, ts(tt, P)],
                                     wo[:, ts(ne, 512)], start=True, stop=True)
                outt = work.tile([P, E], BF16, name=f"outt_{b}_{sci}_{tt}",
                                 tag="outt", bufs=3)
                nc.vector.tensor_copy(out=outt[:], in_=ops[:])
                nc.sync.dma_start(out_d[ds(tok0, P), :], outt[:])

        # ---- emission ----
        # prologue: minimal JIT set for attention chunk (0,0)
        proj_xa(0)
        proj_qk(0, False)
        proj_v_mm(0)
        proj_v_tr(0, (0, 1))
        proj_v_tr(0, (2, 3))
        proj_qk(0, True)
        # morsel queue in deadline order: kv chunks for b0 first, then b1 kv
        # woven with the remaining Q chunks
        for t in range(1, 4):
            work_q.append(lambda t=t: proj_xa(t))
            work_q.append(lambda t=t: proj_qk(t, False))
            work_q.append(lambda t=t: proj_v_mm(t))
            work_q.append(lambda t=t: proj_v_tr(t, (0, 1)))
            work_q.append(lambda t=t: proj_v_tr(t, (2, 3)))
        work_q.append(lambda: proj_qk(1, True))
        for t in range(4, NTCH):
            work_q.append(lambda t=t: proj_xa(t))
            work_q.append(lambda t=t: proj_qk(t, False))
            work_q.append(lambda t=t: proj_v_mm(t))
            work_q.append(lambda t=t: proj_v_tr(t, (0, 1)))
            work_q.append(lambda t=t: proj_v_tr(t, (2, 3)))
            work_q.append(lambda t=t: proj_qk(t - 2, True))
        work_q.append(lambda: proj_qk(6, True))
        work_q.append(lambda: proj_qk(7, True))

        chunks = [(0, s) for s in range(NSC)] + [(1, s) for s in range(NSC)]
        prev = None
        for i, (b, sci) in enumerate(chunks):
            pv = attn_compute(b, sci, npop=2 if i == 0 else 1)
            if prev is not None:
                attn_finish(*prev)
            prev = (b, sci, *pv)
        while work_q:
            work_q.popleft()()
        attn_finish(*prev)
        if DEBUG_DUMP:
            nc.sync.dma_start(dqt_d, QT[:])
            nc.sync.dma_start(dkt_d, KT[:])
            nc.sync.dma_start(dv_d, V.rearrange("p a b -> p (a b)"))
            nc.sync.dma_start(dxa_d, xa.rearrange("p a b -> p (a b)"))

    nc.compile()
    return nc


def _get_nc(use_mask: bool):
    if use_mask not in _nc_cache:
        _nc_cache[use_mask] = _build_nc(use_mask)
    return _nc_cache[use_mask]


def _prep_in_maps(x, attn_mask, in_proj_weight, in_proj_bias, out_proj_weight,
                  lora_a, lora_b, use_mask):
    bf = ml_dtypes.bfloat16

    def wtile(w2d):  # [E, M] -> [P, ET, M] contiguous
        m = w2d.shape[1]
        return np.ascontiguousarray(
            w2d.reshape(ET, P, m).transpose(1, 0, 2)).astype(bf)

    xf = x.reshape(T, E)
    xT = np.ascontiguousarray(
        xf.reshape(NTCH, TCH, ET, P).transpose(3, 0, 2, 1)).astype(bf)
    aT = wtile(lora_a.T)
    maskT = None
    if use_mask:
        maskT = np.ascontiguousarray(attn_mask.transpose(0, 2, 1)).astype(bf)
    in_maps = []
    for c in range(NCORES):
        h0 = 2 * c
        qs = slice(h0 * D, (h0 + 2) * D)
        ks = slice(E + h0 * D, E + (h0 + 2) * D)
        vs = slice(2 * E + h0 * D, 2 * E + (h0 + 2) * D)
        m = {
            "xT": xT,
            "aT": aT,
            "wqT": wtile(in_proj_weight[qs, :].T * SCALE),
            "wkT": wtile(in_proj_weight[ks, :].T),
            "wvT": wtile(in_proj_weight[vs, :].T),
            "bqT": np.ascontiguousarray(lora_b[qs, :].T * SCALE).astype(bf),
            "bkT": np.ascontiguousarray(lora_b[ks, :].T).astype(bf),
            "bvT": np.ascontiguousarray(lora_b[vs, :].T).astype(bf),
            "qb": np.ascontiguousarray((in_proj_bias[qs] * SCALE)[:, None]).astype(np.float32),
            "kb": np.ascontiguousarray(in_proj_bias[ks][:, None]).astype(np.float32),
            "vb": np.ascontiguousarray(in_proj_bias[vs][:, None]).astype(np.float32),
            "woT": np.ascontiguousarray(out_proj_weight[:, h0 * D:(h0 + 2) * D].T).astype(bf),
        }
        if use_mask:
            m["maskT"] = maskT
        in_maps.append(m)
    return in_maps


def kernel(x, attn_mask, in_proj_weight, in_proj_bias, out_proj_weight,
           out_proj_bias, lora_a, lora_b, _trace=False):
    x = np.asarray(x, dtype=np.float32)
    attn_mask = np.asarray(attn_mask, dtype=np.float32)
    in_proj_weight = np.asarray(in_proj_weight, dtype=np.float32)
    in_proj_bias = np.asarray(in_proj_bias, dtype=np.float32)
    out_proj_weight = np.asarray(out_proj_weight, dtype=np.float32)
    out_proj_bias = np.asarray(out_proj_bias, dtype=np.float32)
    lora_a = np.asarray(lora_a, dtype=np.float32)
    lora_b = np.asarray(lora_b, dtype=np.float32)

    use_mask = bool(np.any(attn_mask))
    nc = _get_nc(use_mask)
    in_maps = _prep_in_maps(x, attn_mask, in_proj_weight, in_proj_bias,
                            out_proj_weight, lora_a, lora_b, use_mask)
    res = run_bass_kernel_spmd(nc, in_maps, core_ids=list(range(NCORES)),
                               trace=_trace)
    acc = np.zeros((T, E), np.float32)
    for c in range(NCORES):
        acc += res.results[c]["out"].astype(np.float32)
    acc += out_proj_bias[None, :]
    out = acc.reshape(B, S, E)
    if _trace:
        kernel._last_exec_time_ns = res.exec_time_ns
        kernel._last_trace = (res.instructions_and_trace[1]
                              if res.instructions_and_trace else None)
    return out


# revision 25
# speedup vs baseline: 1.3334x; 1.0979x over previous
"""Trainium2 Bass kernel for nn_Attention_29807073034381.

Multi-head attention (B=2, S=2048, E=1024, H=16, D=64) with LoRA-augmented QKV
projection, sharded 2-heads-per-core across 8 NeuronCores (tensor parallel).

Key choices:
  - LoRA is linear, so the host folds it into the projection weights
    (W_eff = W + lora_b @ lora_a) and slices/transposes per core; the softmax
    scaling is folded into Wq/bias_q. No LoRA compute on device.
  - All device compute in bf16 with fp32 PSUM accumulation (rel-err gate 2e-2;
    measured ~6e-3). Host passes x pre-transposed/pre-tiled so every matmul
    contraction sits on the SBUF partition axis and every DMA line is 8KB.
  - projection: QT/KT [128, T] (head-dim on partitions, weight-stationary);
    V computed as VT then PE-transposed into [tok, dv] tiles augmented with a
    ones column; Q/K bias added as a per-partition scalar at PSUM-copy time.
  - attention per (b, si-chunk of 512): scoresT tiles [sj=128, si=512] for
    both heads packed into one [128, 1024] PSUM tile via K=64 row-packed
    matmuls (tile_position (0,0)/(64,0)); one ACT exp per sj-tile
    (PSUM f32 -> SBUF bf16); PV matmuls against ones-augmented V accumulate
    attnoutT and the softmax denominator in one PSUM group.
  - normalize via reciprocal_approx_fast + gpsimd partition_broadcast,
    multiplied in during the PSUM->SBUF copy (bf16 attnT [hd=128, si]);
    V-bias added post-normalize (P@(V + 1*vb) = PV + denom*vb).
  - out_proj: partial = attnT.T @ woT slice -> [tok, E] bf16 partial output;
    host sums the 8 partials in fp32 and adds out_proj_bias.

Scheduling: emission order is a chunk-level software pipeline. Projection
work is chopped into ~2-4us "morsels" on a queue; the attention sj-loop pops
one morsel BEFORE each sj iteration (so producers precede consumers in PE
program order — the queue is deadline-ordered), keeping ScalarE (exp) fed
while the PE fills its slack with projection work. finish(i-1) (normalize +
out_proj) is emitted after compute(i) so the PE never idles long enough to
re-throttle (HAM).

attn_mask is all-zeros in this problem's setup_inputs; a masked variant of
the graph (maskT added to scores pre-exp) is built only if a nonzero mask
ever shows up.
"""

import numpy as np
import ml_dtypes
from collections import deque
from contextlib import ExitStack

import concourse.bass as bass
import concourse.bacc as bacc
import concourse.tile as tile
import concourse.mybir as mybir
from concourse.bass_utils import run_bass_kernel_spmd
from concourse.bass import ts, ds
from concourse.masks import make_identity

BF16 = mybir.dt.bfloat16
F32 = mybir.dt.float32

P = 128
E = 1024
H = 16
D = 64
B = 2
S = 2048
T = B * S            # 4096 tokens
ET = E // P          # 8 e-tiles
TCH = 512            # projection token chunk
NTCH = T // TCH      # 8
SC = 512             # attention si chunk
NSC = S // SC        # 4 per batch
SJT = S // P         # 16 sj tiles per batch
NCORES = 8
SCALE = float(D) ** -0.5

_nc_cache = {}


def _build_nc(use_mask: bool):
    nc = bacc.Bacc("TRN2", target_bir_lowering=False, debug=False,
                   num_devices=NCORES)
    xT_d = nc.dram_tensor("xT", [P, NTCH, ET, TCH], BF16,
                          kind="ExternalInput").ap()
    wq_d = nc.dram_tensor("wqT", [P, ET, P], BF16, kind="ExternalInput").ap()
    wk_d = nc.dram_tensor("wkT", [P, ET, P], BF16, kind="ExternalInput").ap()
    wv_d = nc.dram_tensor("wvT", [P, ET, P], BF16, kind="ExternalInput").ap()
    qb_d = nc.dram_tensor("qb", [P, 1], F32, kind="ExternalInput").ap()
    kb_d = nc.dram_tensor("kb", [P, 1], F32, kind="ExternalInput").ap()
    vb_d = nc.dram_tensor("vb", [P, 1], F32, kind="ExternalInput").ap()
    wo_d = nc.dram_tensor("woT", [P, E], BF16, kind="ExternalInput").ap()
    mask_d = None
    if use_mask:
        mask_d = nc.dram_tensor("maskT", [B, S, S], BF16,
                                kind="ExternalInput").ap()
    out_d = nc.dram_tensor("out", [T, E], BF16, kind="ExternalOutput").ap()

    with tile.TileContext(nc) as tc, ExitStack() as ctx:
        persist = ctx.enter_context(tc.tile_pool(name="persist", bufs=1))
        work = ctx.enter_context(tc.tile_pool(name="work", bufs=2))
        expp = ctx.enter_context(tc.tile_pool(name="expp", bufs=6))
        psum = ctx.enter_context(tc.tile_pool(name="psum", bufs=2, space="PSUM"))

        # ---- persistent SBUF tensors ----
        xT = persist.tile([P, NTCH, ET, TCH], BF16, name="xT_sb", tag="xT_sb")
        wq = persist.tile([P, ET, P], BF16, name="wq_sb", tag="wq_sb")
        wk = persist.tile([P, ET, P], BF16, name="wk_sb", tag="wk_sb")
        wv = persist.tile([P, ET, P], BF16, name="wv_sb", tag="wv_sb")
        qb = persist.tile([P, 1], F32, name="qb_sb", tag="qb_sb")
        kb = persist.tile([P, 1], F32, name="kb_sb", tag="kb_sb")
        vb = persist.tile([P, 1], F32, name="vb_sb", tag="vb_sb")
        wo = persist.tile([P, E], BF16, name="wo_sb", tag="wo_sb")
        ident = persist.tile([P, P], BF16, name="ident_sb", tag="ident_sb")
        QT = persist.tile([P, T], BF16, name="QT_sb", tag="QT_sb")
        KT = persist.tile([P, T], BF16, name="KT_sb", tag="KT_sb")
        V = persist.tile([P, T // P, 2 * (D + 1)], BF16, name="V_sb", tag="V_sb")

        nc.sync.dma_start(wq[:], wq_d)
        nc.sync.dma_start(wk[:], wk_d)
        nc.sync.dma_start(wv[:], wv_d)
        nc.sync.dma_start(qb[:], qb_d)
        nc.sync.dma_start(kb[:], kb_d)
        nc.sync.dma_start(vb[:], vb_d)
        nc.sync.dma_start(wo[:], wo_d)
        for t in range(NTCH):
            nc.sync.dma_start(xT[:, t], xT_d[:, t])

        make_identity(nc, ident[:])
        # ones columns for the softmax-denominator augmentation of V
        nc.vector.memset(V[:, :, D:D + 1], 1.0)
        nc.vector.memset(V[:, :, 2 * D + 1:2 * D + 2], 1.0)

        # ---- stage A morsels ----
        def proj_qk(t, is_q):
            w, bcol, dst = (wq, qb, QT) if is_q else (wk, kb, KT)
            nm = "q" if is_q else "k"
            ps = psum.tile([P, TCH], F32, name=f"{nm}_ps_{t}", tag="a")
            for e in range(ET):
                nc.tensor.matmul(ps[:], w[:, e, :], xT[:, t, e, :],
                                 start=(e == 0), stop=(e == ET - 1))
            nc.vector.tensor_scalar_add(dst[:, ts(t, TCH)], ps[:], bcol[:])

        vt_tiles = {}

        def proj_v_mm(t):
            ps = psum.tile([P, TCH], F32, name=f"vt_ps_{t}", tag="a")
            for e in range(ET):
                nc.tensor.matmul(ps[:], wv[:, e, :], xT[:, t, e, :],
                                 start=(e == 0), stop=(e == ET - 1))
            vt_sb = work.tile([P, TCH], BF16, name=f"vt_sb_{t}", tag="vt",
                              bufs=2)
            nc.vector.tensor_copy(out=vt_sb[:], in_=ps[:])
            vt_tiles[t] = vt_sb

        def proj_v_tr(t, s4s):
            vt_sb = vt_tiles[t]
            for s4 in s4s:
                jt = t * (TCH // P) + s4
                pt = psum.tile([P, P], BF16, name=f"vtr_ps_{jt}", tag="a")
                nc.tensor.transpose(pt[:], vt_sb[:, ds(s4 * P, P)], ident[:])
                nc.vector.tensor_copy(
                    out=V[:, jt].rearrange("p (g c) -> p g c", g=2)[:, :, 0:D],
                    in_=pt.rearrange("p (g c) -> p g c", g=2))

        # ---- stage B: attention sj-loop; pops one morsel per sj tile ----
        work_q = deque()

        def attn_compute(b, sci, npop=1):
            si0 = b * S + sci * SC
            pvA = psum.tile([D + 1, SC], F32, name=f"pvA_{b}_{sci}", tag="pv")
            pvB = psum.tile([D + 1, SC], F32, name=f"pvB_{b}_{sci}", tag="pv")
            for sjt in range(SJT):
                # emit queued projection morsels BEFORE this iteration so
                # their tiles are written earlier in PE program order than
                # the scores/PV matmuls that read them (deadline-ordered)
                for _ in range(npop):
                    if work_q:
                        work_q.popleft()()
                jt = b * SJT + sjt
                scs = psum.tile([P, 2 * SC], F32, name=f"scs_{b}_{sci}_{sjt}",
                                tag="sc")
                nc.tensor.matmul(scs[:, 0:SC], KT[0:D, ds(jt * P, P)],
                                 QT[0:D, ds(si0, SC)], start=True, stop=True,
                                 tile_position=(0, 0))
                nc.tensor.matmul(scs[:, SC:2 * SC], KT[D:P, ds(jt * P, P)],
                                 QT[D:P, ds(si0, SC)], start=True, stop=True,
                                 tile_position=(64, 0))
                if use_mask:
                    mt = work.tile([P, SC], BF16, name=f"mt_{b}_{sci}_{sjt}",
                                   tag="mask", bufs=3)
                    nc.sync.dma_start(
                        mt[:], mask_d[b, ds(sjt * P, P), ds(sci * SC, SC)])
                    nc.vector.tensor_tensor(
                        out=scs.rearrange("p (g c) -> p g c", g=2),
                        in0=scs.rearrange("p (g c) -> p g c", g=2),
                        in1=mt[:, None, :].to_broadcast([P, 2, SC]),
                        op=mybir.AluOpType.add)
                expab = expp.tile([P, 2 * SC], BF16, name=f"ex_{b}_{sci}_{sjt}",
                                  tag="exp")
                nc.scalar.activation(expab[:], scs[:],
                                     mybir.ActivationFunctionType.Exp)
                nc.tensor.matmul(pvA[:], V[:, jt, 0:D + 1], expab[:, 0:SC],
                                 start=(sjt == 0), stop=(sjt == SJT - 1))
                nc.tensor.matmul(pvB[:], V[:, jt, D + 1:2 * (D + 1)],
                                 expab[:, SC:2 * SC],
                                 start=(sjt == 0), stop=(sjt == SJT - 1))
            return pvA, pvB

        # ---- normalize + out_proj for a finished (b, si-chunk) ----
        def attn_finish(b, sci, pvA, pvB):
            si0 = b * S + sci * SC
            attnT = work.tile([P, SC], BF16, name=f"attnT_{b}_{sci}",
                              tag="attnT")
            for hh, pv in ((0, pvA), (1, pvB)):
                den = work.tile([1, SC], F32, name=f"den_{b}_{sci}_{hh}",
                                tag="den", bufs=4)
                nc.vector.tensor_copy(out=den[:], in_=pv[D:D + 1, :])
                rec = work.tile([1, SC], F32, name=f"rec_{b}_{sci}_{hh}",
                                tag="rec", bufs=4)
                nc.vector.reciprocal_approx_fast(out=rec[:], in_=den[:])
                bc = work.tile([D, SC], F32, name=f"bc_{b}_{sci}_{hh}",
                               tag="bc", bufs=4)
                nc.gpsimd.partition_broadcast(bc[:], rec[:])
                nc.vector.tensor_tensor(out=attnT[hh * D:(hh + 1) * D, :],
                                        in0=pv[0:D, :], in1=bc[:],
                                        op=mybir.AluOpType.mult)
                # V-bias: P@(V + 1*vb) = PV + denom*vb -> add vb post-normalize
                nc.vector.tensor_scalar_add(attnT[hh * D:(hh + 1) * D, :],
                                            attnT[hh * D:(hh + 1) * D, :],
                                            vb[hh * D:(hh + 1) * D, :])
            for tt in range(SC // P):
                tok0 = si0 + tt * P
                ops = psum.tile([P, E], F32, name=f"o_ps_{b}_{sci}_{tt}",
                                tag="sc")
                for ne in range(E // 512):
                    nc.tensor.matmul(ops[:, ts(ne, 512)], attnT[:, ts(tt, P)],
                                     wo[:, ts(ne, 512)], start=True, stop=True)
                outt = work.tile([P, E], BF16, name=f"outt_{b}_{sci}_{tt}",
                                 tag="outt", bufs=3)
                nc.vector.tensor_copy(out=outt[:], in_=ops[:])
                nc.sync.dma_start(out_d[ds(tok0, P), :], outt[:])

        # ---- emission ----
        # prologue: minimal JIT set for attention chunk (0,0)
        proj_qk(0, False)
        proj_v_mm(0)
        proj_v_tr(0, (0, 1))
        proj_v_tr(0, (2, 3))
        proj_qk(0, True)
        # morsel queue in deadline order (1 pop/sjt meets every deadline:
        # kv chunk c lands at pops 4c-3..4c, needed at sj tile 4c)
        for t in range(1, 4):
            work_q.append(lambda t=t: proj_qk(t, False))
            work_q.append(lambda t=t: proj_v_mm(t))
            work_q.append(lambda t=t: proj_v_tr(t, (0, 1)))
            work_q.append(lambda t=t: proj_v_tr(t, (2, 3)))
        work_q.append(lambda: proj_qk(1, True))
        for t in range(4, NTCH):
            work_q.append(lambda t=t: proj_qk(t, False))
            work_q.append(lambda t=t: proj_v_mm(t))
            work_q.append(lambda t=t: proj_v_tr(t, (0, 1)))
            work_q.append(lambda t=t: proj_v_tr(t, (2, 3)))
            work_q.append(lambda t=t: proj_qk(t - 2, True))
        work_q.append(lambda: proj_qk(6, True))
        work_q.append(lambda: proj_qk(7, True))

        chunks = [(0, s) for s in range(NSC)] + [(1, s) for s in range(NSC)]
        prev = None
        for b, sci in chunks:
            pv = attn_compute(b, sci)
            if prev is not None:
                attn_finish(*prev)
            prev = (b, sci, *pv)
        while work_q:
            work_q.popleft()()
        attn_finish(*prev)

    nc.compile()
    return nc


def _get_nc(use_mask: bool):
    if use_mask not in _nc_cache:
        _nc_cache[use_mask] = _build_nc(use_mask)
    return _nc_cache[use_mask]


def _prep_in_maps(x, attn_mask, in_proj_weight, in_proj_bias, out_proj_weight,
                  lora_a, lora_b, use_mask):
    bf = ml_dtypes.bfloat16

    def wtile(w2d):  # [E, M] -> [P, ET, M] contiguous
        m = w2d.shape[1]
        return np.ascontiguousarray(
            w2d.reshape(ET, P, m).transpose(1, 0, 2)).astype(bf)

    xf = x.reshape(T, E)
    xT = np.ascontiguousarray(
        xf.reshape(NTCH, TCH, ET, P).transpose(3, 0, 2, 1)).astype(bf)
    # fold the (linear) LoRA delta into the projection weights
    w_eff = in_proj_weight + lora_b @ lora_a
    maskT = None
    if use_mask:
        maskT = np.ascontiguousarray(attn_mask.transpose(0, 2, 1)).astype(bf)
    in_maps = []
    for c in range(NCORES):
        h0 = 2 * c
        qs = slice(h0 * D, (h0 + 2) * D)
        ks = slice(E + h0 * D, E + (h0 + 2) * D)
        vs = slice(2 * E + h0 * D, 2 * E + (h0 + 2) * D)
        m = {
            "xT": xT,
            "wqT": wtile(w_eff[qs, :].T * SCALE),
            "wkT": wtile(w_eff[ks, :].T),
            "wvT": wtile(w_eff[vs, :].T),
            "qb": np.ascontiguousarray((in_proj_bias[qs] * SCALE)[:, None]).astype(np.float32),
            "kb": np.ascontiguousarray(in_proj_bias[ks][:, None]).astype(np.float32),
            "vb": np.ascontiguousarray(in_proj_bias[vs][:, None]).astype(np.float32),
            "woT": np.ascontiguousarray(out_proj_weight[:, h0 * D:(h0 + 2) * D].T).astype(bf),
        }
        if use_mask:
            m["maskT"] = maskT
        in_maps.append(m)
    return in_maps


def kernel(x, attn_mask, in_proj_weight, in_proj_bias, out_proj_weight,
           out_proj_bias, lora_a, lora_b, _trace=False):
    x = np.asarray(x, dtype=np.float32)
    attn_mask = np.asarray(attn_mask, dtype=np.float32)
    in_proj_weight = np.asarray(in_proj_weight, dtype=np.float32)
    in_proj_bias = np.asarray(in_proj_bias, dtype=np.float32)
    out_proj_weight = np.asarray(out_proj_weight, dtype=np.float32)
    out_proj_bias = np.asarray(out_proj_bias, dtype=np.float32)
    lora_a = np.asarray(lora_a, dtype=np.float32)
    lora_b = np.asarray(lora_b, dtype=np.float32)

    use_mask = bool(np.any(attn_mask))
    nc = _get_nc(use_mask)
    in_maps = _prep_in_maps(x, attn_mask, in_proj_weight, in_proj_bias,
                            out_proj_weight, lora_a, lora_b, use_mask)
    res = run_bass_kernel_spmd(nc, in_maps, core_ids=list(range(NCORES)),
                               trace=_trace)
    acc = np.zeros((T, E), np.float32)
    for c in range(NCORES):
        acc += res.results[c]["out"].astype(np.float32)
    acc += out_proj_bias[None, :]
    out = acc.reshape(B, S, E)
    if _trace:
        kernel._last_exec_time_ns = res.exec_time_ns
        kernel._last_trace = (res.instructions_and_trace[1]
                              if res.instructions_and_trace else None)
    return out


# revision 29
# speedup vs baseline: 1.4014x; 1.0510x over previous
"""Trainium2 Bass kernel for nn_Attention_29807073034381.

Multi-head attention (B=2, S=2048, E=1024, H=16, D=64) with LoRA-augmented QKV
projection, sharded 2-heads-per-core across 8 NeuronCores (tensor parallel).

Key choices:
  - LoRA is linear, so the host folds it into the projection weights
    (W_eff = W + lora_b @ lora_a) and slices/transposes per core; the softmax
    scaling is folded into Wq/bias_q. No LoRA compute on device.
  - All device compute in bf16 with fp32 PSUM accumulation (rel-err gate 2e-2;
    measured ~6e-3). Host passes x pre-transposed/pre-tiled so every matmul
    contraction sits on the SBUF partition axis and every DMA line is 8KB.
  - projection: QT/KT [128, T] (head-dim on partitions, weight-stationary);
    V computed as VT then PE-transposed into [tok, dv] tiles augmented with a
    ones column; Q/K bias added as a per-partition scalar at PSUM-copy time.
  - attention per (b, si-chunk of 512): scoresT tiles [sj=128, si=512] for
    both heads packed into one [128, 1024] PSUM tile via K=64 row-packed
    matmuls (tile_position (0,0)/(64,0)); one ACT exp per sj-tile
    (PSUM f32 -> SBUF bf16); PV matmuls against ones-augmented V accumulate
    attnoutT and the softmax denominator in one PSUM group.
  - normalize via reciprocal_approx_fast + gpsimd partition_broadcast,
    multiplied in during the PSUM->SBUF copy (bf16 attnT [hd=128, si]);
    V-bias added post-normalize (P@(V + 1*vb) = PV + denom*vb).
  - out_proj: partial = attnT.T @ woT slice -> [tok, E] bf16 partial output;
    host sums the 8 partials in fp32 and adds out_proj_bias.

Scheduling: emission order is a chunk-level software pipeline. Projection
work is chopped into ~2-4us "morsels" on a queue; the attention sj-loop pops
one morsel BEFORE each sj iteration (so producers precede consumers in PE
program order — the queue is deadline-ordered), keeping ScalarE (exp) fed
while the PE fills its slack with projection work. finish(i-1) (normalize +
out_proj) is emitted after compute(i) so the PE never idles long enough to
re-throttle (HAM).

attn_mask is all-zeros in this problem's setup_inputs; a masked variant of
the graph (maskT added to scores pre-exp) is built only if a nonzero mask
ever shows up.
"""

import numpy as np
import ml_dtypes
from collections import deque
from contextlib import ExitStack

import concourse.bass as bass
import concourse.bacc as bacc
import concourse.tile as tile
import concourse.mybir as mybir
from concourse.bass_utils import run_bass_kernel_spmd
from concourse.bass import ts, ds
from concourse.masks import make_identity

BF16 = mybir.dt.bfloat16
F32 = mybir.dt.float32

P = 128
E = 1024
H = 16
D = 64
B = 2
S = 2048
T = B * S            # 4096 tokens
ET = E // P          # 8 e-tiles
TCH = 512            # projection token chunk
NTCH = T // TCH      # 8
SC = 512             # attention si chunk
NSC = S // SC        # 4 per batch
SJT = S // P         # 16 sj tiles per batch
NCORES = 8
SCALE = float(D) ** -0.5

_nc_cache = {}


def _build_nc(use_mask: bool):
    nc = bacc.Bacc("TRN2", target_bir_lowering=False, debug=False,
                   num_devices=NCORES)
    xT_d = nc.dram_tensor("xT", [P, NTCH, ET, TCH], BF16,
                          kind="ExternalInput").ap()
    wq_d = nc.dram_tensor("wqT", [P, ET, P], BF16, kind="ExternalInput").ap()
    wk_d = nc.dram_tensor("wkT", [P, ET, P], BF16, kind="ExternalInput").ap()
    wv_d = nc.dram_tensor("wvT", [P, ET, P], BF16, kind="ExternalInput").ap()
    qb_d = nc.dram_tensor("qb", [P, 1], F32, kind="ExternalInput").ap()
    kb_d = nc.dram_tensor("kb", [P, 1], F32, kind="ExternalInput").ap()
    vb_d = nc.dram_tensor("vb", [P, 1], F32, kind="ExternalInput").ap()
    wo_d = nc.dram_tensor("woT", [P, E], BF16, kind="ExternalInput").ap()
    mask_d = None
    if use_mask:
        mask_d = nc.dram_tensor("maskT", [B, S, S], BF16,
                                kind="ExternalInput").ap()
    out_d = nc.dram_tensor("out", [T, E], BF16, kind="ExternalOutput").ap()

    with tile.TileContext(nc) as tc, ExitStack() as ctx:
        persist = ctx.enter_context(tc.tile_pool(name="persist", bufs=1))
        work = ctx.enter_context(tc.tile_pool(name="work", bufs=2))
        expp = ctx.enter_context(tc.tile_pool(name="expp", bufs=6))
        psum = ctx.enter_context(tc.tile_pool(name="psum", bufs=2, space="PSUM"))

        # ---- persistent SBUF tensors ----
        xT = persist.tile([P, NTCH, ET, TCH], BF16, name="xT_sb", tag="xT_sb")
        wq = persist.tile([P, ET, P], BF16, name="wq_sb", tag="wq_sb")
        wk = persist.tile([P, ET, P], BF16, name="wk_sb", tag="wk_sb")
        wv = persist.tile([P, ET, P], BF16, name="wv_sb", tag="wv_sb")
        qb = persist.tile([P, 1], F32, name="qb_sb", tag="qb_sb")
        kb = persist.tile([P, 1], F32, name="kb_sb", tag="kb_sb")
        vb = persist.tile([P, 1], F32, name="vb_sb", tag="vb_sb")
        wo = persist.tile([P, E], BF16, name="wo_sb", tag="wo_sb")
        ident = persist.tile([P, P], BF16, name="ident_sb", tag="ident_sb")
        QT = persist.tile([P, T], BF16, name="QT_sb", tag="QT_sb")
        KT = persist.tile([P, T], BF16, name="KT_sb", tag="KT_sb")
        V = persist.tile([P, T // P, 2 * (D + 1)], BF16, name="V_sb", tag="V_sb")

        nc.sync.dma_start(wq[:], wq_d)
        nc.sync.dma_start(wk[:], wk_d)
        nc.sync.dma_start(wv[:], wv_d)
        nc.sync.dma_start(qb[:], qb_d)
        nc.sync.dma_start(kb[:], kb_d)
        nc.sync.dma_start(vb[:], vb_d)
        nc.sync.dma_start(wo[:], wo_d)
        for t in range(NTCH):
            nc.sync.dma_start(xT[:, t], xT_d[:, t])

        make_identity(nc, ident[:])
        # ones columns for the softmax-denominator augmentation of V
        nc.vector.memset(V[:, :, D:D + 1], 1.0)
        nc.vector.memset(V[:, :, 2 * D + 1:2 * D + 2], 1.0)

        # ---- stage A morsels ----
        def proj_qk(t, is_q):
            w, bcol, dst = (wq, qb, QT) if is_q else (wk, kb, KT)
            nm = "q" if is_q else "k"
            ps = psum.tile([P, TCH], F32, name=f"{nm}_ps_{t}", tag="a")
            for e in range(ET):
                nc.tensor.matmul(ps[:], w[:, e, :], xT[:, t, e, :],
                                 start=(e == 0), stop=(e == ET - 1))
            nc.vector.tensor_scalar_add(dst[:, ts(t, TCH)], ps[:], bcol[:])

        vt_tiles = {}

        def proj_v_mm(t):
            ps = psum.tile([P, TCH], F32, name=f"vt_ps_{t}", tag="a")
            for e in range(ET):
                nc.tensor.matmul(ps[:], wv[:, e, :], xT[:, t, e, :],
                                 start=(e == 0), stop=(e == ET - 1))
            vt_sb = work.tile([P, TCH], BF16, name=f"vt_sb_{t}", tag="vt",
                              bufs=2)
            nc.vector.tensor_copy(out=vt_sb[:], in_=ps[:])
            vt_tiles[t] = vt_sb

        def proj_v_tr(t, s4s):
            vt_sb = vt_tiles[t]
            for s4 in s4s:
                jt = t * (TCH // P) + s4
                pt = psum.tile([P, P], BF16, name=f"vtr_ps_{jt}", tag="a")
                nc.tensor.transpose(pt[:], vt_sb[:, ds(s4 * P, P)], ident[:])
                nc.vector.tensor_copy(
                    out=V[:, jt].rearrange("p (g c) -> p g c", g=2)[:, :, 0:D],
                    in_=pt.rearrange("p (g c) -> p g c", g=2))

        # ---- stage B: attention sj-loop; pops one morsel per sj tile ----
        work_q = deque()

        def attn_compute(b, sci, npop=1):
            si0 = b * S + sci * SC
            pvA = psum.tile([D + 1, SC], F32, name=f"pvA_{b}_{sci}", tag="pv")
            pvB = psum.tile([D + 1, SC], F32, name=f"pvB_{b}_{sci}", tag="pv")

            def pv_mms(sjt):
                jt = b * SJT + sjt
                expab = exp_tiles[sjt]
                nc.tensor.matmul(pvA[:], V[:, jt, 0:D + 1], expab[:, 0:SC],
                                 start=(sjt == 0), stop=(sjt == SJT - 1))
                nc.tensor.matmul(pvB[:], V[:, jt, D + 1:2 * (D + 1)],
                                 expab[:, SC:2 * SC],
                                 start=(sjt == 0), stop=(sjt == SJT - 1))

            exp_tiles = {}
            for sjt in range(SJT):
                # emit queued projection morsels BEFORE this iteration so
                # their tiles are written earlier in PE program order than
                # the scores/PV matmuls that read them (deadline-ordered)
                for _ in range(npop):
                    if work_q:
                        work_q.popleft()()
                jt = b * SJT + sjt
                scs = psum.tile([P, 2 * SC], F32, name=f"scs_{b}_{sci}_{sjt}",
                                tag="sc")
                nc.tensor.matmul(scs[:, 0:SC], KT[0:D, ds(jt * P, P)],
                                 QT[0:D, ds(si0, SC)], start=True, stop=True,
                                 tile_position=(0, 0))
                nc.tensor.matmul(scs[:, SC:2 * SC], KT[D:P, ds(jt * P, P)],
                                 QT[D:P, ds(si0, SC)], start=True, stop=True,
                                 tile_position=(64, 0))
                if use_mask:
                    mt = work.tile([P, SC], BF16, name=f"mt_{b}_{sci}_{sjt}",
                                   tag="mask", bufs=3)
                    nc.sync.dma_start(
                        mt[:], mask_d[b, ds(sjt * P, P), ds(sci * SC, SC)])
                    nc.vector.tensor_tensor(
                        out=scs.rearrange("p (g c) -> p g c", g=2),
                        in0=scs.rearrange("p (g c) -> p g c", g=2),
                        in1=mt[:, None, :].to_broadcast([P, 2, SC]),
                        op=mybir.AluOpType.add)
                expab = expp.tile([P, 2 * SC], BF16, name=f"ex_{b}_{sci}_{sjt}",
                                  tag="exp")
                nc.scalar.activation(expab[:], scs[:],
                                     mybir.ActivationFunctionType.Exp)
                exp_tiles[sjt] = expab
                # PV trails by one sj tile: its exp-wait is then pre-cleared
                # when the PE reaches it, so the LDWEIGHTS pipelines behind
                # the scores matmul stream instead of serializing post-wait
                if sjt > 0:
                    pv_mms(sjt - 1)
            pv_mms(SJT - 1)
            return pvA, pvB

        # ---- normalize + out_proj for a finished (b, si-chunk) ----
        def out_proj(b, sci, attnT, tts):
            si0 = b * S + sci * SC
            for tt in tts:
                tok0 = si0 + tt * P
                ops = psum.tile([P, E], F32, name=f"o_ps_{b}_{sci}_{tt}",
                                tag="sc")
                for ne in range(E // 512):
                    nc.tensor.matmul(ops[:, ts(ne, 512)], attnT[:, ts(tt, P)],
                                     wo[:, ts(ne, 512)], start=True, stop=True)
                outt = work.tile([P, E], BF16, name=f"outt_{b}_{sci}_{tt}",
                                 tag="outt", bufs=3)
                nc.vector.tensor_copy(out=outt[:], in_=ops[:])
                nc.sync.dma_start(out_d[ds(tok0, P), :], outt[:])

        def attn_finish(b, sci, pvA, pvB):
            attnT = work.tile([P, SC], BF16, name=f"attnT_{b}_{sci}",
                              tag="attnT", bufs=3)
            for hh, pv in ((0, pvA), (1, pvB)):
                den = work.tile([1, SC], F32, name=f"den_{b}_{sci}_{hh}",
                                tag="den", bufs=4)
                nc.vector.tensor_copy(out=den[:], in_=pv[D:D + 1, :])
                rec = work.tile([1, SC], F32, name=f"rec_{b}_{sci}_{hh}",
                                tag="rec", bufs=4)
                nc.vector.reciprocal_approx_fast(out=rec[:], in_=den[:])
                bc = work.tile([D, SC], F32, name=f"bc_{b}_{sci}_{hh}",
                               tag="bc", bufs=4)
                nc.gpsimd.partition_broadcast(bc[:], rec[:])
                nc.vector.tensor_tensor(out=attnT[hh * D:(hh + 1) * D, :],
                                        in0=pv[0:D, :], in1=bc[:],
                                        op=mybir.AluOpType.mult)
                # V-bias: P@(V + 1*vb) = PV + denom*vb -> add vb post-normalize
                nc.vector.tensor_scalar_add(attnT[hh * D:(hh + 1) * D, :],
                                            attnT[hh * D:(hh + 1) * D, :],
                                            vb[hh * D:(hh + 1) * D, :])
            # out_proj goes on the morsel queue (popped during the next
            # chunk's sj-loop) so it doesn't block the PE at chunk boundary
            work_q.append(lambda: out_proj(b, sci, attnT, (0, 1)))
            work_q.append(lambda: out_proj(b, sci, attnT, (2, 3)))

        # ---- emission ----
        # prologue: minimal JIT set for attention chunk (0,0)
        proj_qk(0, False)
        proj_v_mm(0)
        proj_v_tr(0, (0, 1))
        proj_v_tr(0, (2, 3))
        proj_qk(0, True)
        # morsel queue in deadline order (1 pop/sjt meets every deadline:
        # kv chunk c lands at pops 4c-3..4c, needed at sj tile 4c)
        for t in range(1, 4):
            work_q.append(lambda t=t: proj_qk(t, False))
            work_q.append(lambda t=t: proj_v_mm(t))
            work_q.append(lambda t=t: proj_v_tr(t, (0, 1)))
            work_q.append(lambda t=t: proj_v_tr(t, (2, 3)))
        work_q.append(lambda: proj_qk(1, True))
        for t in range(4, NTCH):
            work_q.append(lambda t=t: proj_qk(t, False))
            work_q.append(lambda t=t: proj_v_mm(t))
            work_q.append(lambda t=t: proj_v_tr(t, (0, 1)))
            work_q.append(lambda t=t: proj_v_tr(t, (2, 3)))
            work_q.append(lambda t=t: proj_qk(t - 2, True))
        work_q.append(lambda: proj_qk(6, True))
        work_q.append(lambda: proj_qk(7, True))

        chunks = [(0, s) for s in range(NSC)] + [(1, s) for s in range(NSC)]
        prev = None
        for b, sci in chunks:
            pv = attn_compute(b, sci)
            if prev is not None:
                attn_finish(*prev)
            prev = (b, sci, *pv)
        attn_finish(*prev)
        while work_q:
            work_q.popleft()()

    nc.compile()
    return nc


def _get_nc(use_mask: bool):
    if use_mask not in _nc_cache:
        _nc_cache[use_mask] = _build_nc(use_mask)
    return _nc_cache[use_mask]


def _prep_in_maps(x, attn_mask, in_proj_weight, in_proj_bias, out_proj_weight,
                  lora_a, lora_b, use_mask):
    bf = ml_dtypes.bfloat16

    def wtile(w2d):  # [E, M] -> [P, ET, M] contiguous
        m = w2d.shape[1]
        return np.ascontiguousarray(
            w2d.reshape(ET, P, m).transpose(1, 0, 2)).astype(bf)

    xf = x.reshape(T, E)
    xT = np.ascontiguousarray(
        xf.reshape(NTCH, TCH, ET, P).transpose(3, 0, 2, 1)).astype(bf)
    # fold the (linear) LoRA delta into the projection weights
    w_eff = in_proj_weight + lora_b @ lora_a
    maskT = None
    if use_mask:
        maskT = np.ascontiguousarray(attn_mask.transpose(0, 2, 1)).astype(bf)
    in_maps = []
    for c in range(NCORES):
        h0 = 2 * c
        qs = slice(h0 * D, (h0 + 2) * D)
        ks = slice(E + h0 * D, E + (h0 + 2) * D)
        vs = slice(2 * E + h0 * D, 2 * E + (h0 + 2) * D)
        m = {
            "xT": xT,
            "wqT": wtile(w_eff[qs, :].T * SCALE),
            "wkT": wtile(w_eff[ks, :].T),
            "wvT": wtile(w_eff[vs, :].T),
            "qb": np.ascontiguousarray((in_proj_bias[qs] * SCALE)[:, None]).astype(np.float32),
            "kb": np.ascontiguousarray(in_proj_bias[ks][:, None]).astype(np.float32),
            "vb": np.ascontiguousarray(in_proj_bias[vs][:, None]).astype(np.float32),
            "woT": np.ascontiguousarray(out_proj_weight[:, h0 * D:(h0 + 2) * D].T).astype(bf),
        }
        if use_mask:
            m["maskT"] = maskT
        in_maps.append(m)
    return in_maps


def kernel(x, attn_mask, in_proj_weight, in_proj_bias, out_proj_weight,
           out_proj_bias, lora_a, lora_b, _trace=False):
    x = np.asarray(x, dtype=np.float32)
    attn_mask = np.asarray(attn_mask, dtype=np.float32)
    in_proj_weight = np.asarray(in_proj_weight, dtype=np.float32)
    in_proj_bias = np.asarray(in_proj_bias, dtype=np.float32)
    out_proj_weight = np.asarray(out_proj_weight, dtype=np.float32)
    out_proj_bias = np.asarray(out_proj_bias, dtype=np.float32)
    lora_a = np.asarray(lora_a, dtype=np.float32)
    lora_b = np.asarray(lora_b, dtype=np.float32)

    use_mask = bool(np.any(attn_mask))
    nc = _get_nc(use_mask)
    in_maps = _prep_in_maps(x, attn_mask, in_proj_weight, in_proj_bias,
                            out_proj_weight, lora_a, lora_b, use_mask)
    res = run_bass_kernel_spmd(nc, in_maps, core_ids=list(range(NCORES)),
                               trace=_trace)
    acc = np.zeros((T, E), np.float32)
    for c in range(NCORES):
        acc += res.results[c]["out"].astype(np.float32)
    acc += out_proj_bias[None, :]
    out = acc.reshape(B, S, E)
    if _trace:
        kernel._last_exec_time_ns = res.exec_time_ns
        kernel._last_trace = (res.instructions_and_trace[1]
                              if res.instructions_and_trace else None)
    return out


# revision 31
# speedup vs baseline: 1.4323x; 1.0221x over previous
"""Trainium2 Bass kernel for nn_Attention_29807073034381.

Multi-head attention (B=2, S=2048, E=1024, H=16, D=64) with LoRA-augmented QKV
projection, sharded 2-heads-per-core across 8 NeuronCores (tensor parallel).

Key choices:
  - LoRA is linear, so the host folds it into the projection weights
    (W_eff = W + lora_b @ lora_a) and slices/transposes per core; the softmax
    scaling is folded into Wq/bias_q. No LoRA compute on device.
  - All device compute in bf16 with fp32 PSUM accumulation (rel-err gate 2e-2;
    measured ~6e-3). Host passes x pre-transposed/pre-tiled so every matmul
    contraction sits on the SBUF partition axis and every DMA line is 8KB.
  - projection: QT/KT [128, T] (head-dim on partitions, weight-stationary);
    V computed as VT then PE-transposed into [tok, dv] tiles augmented with a
    ones column; Q/K bias added as a per-partition scalar at PSUM-copy time.
  - attention per (b, si-chunk of 512): scoresT tiles [sj=128, si=512] for
    both heads packed into one [128, 1024] PSUM tile via K=64 row-packed
    matmuls (tile_position (0,0)/(64,0)); one ACT exp per sj-tile
    (PSUM f32 -> SBUF bf16); PV matmuls against ones-augmented V accumulate
    attnoutT and the softmax denominator in one PSUM group.
  - normalize via reciprocal_approx_fast + gpsimd partition_broadcast,
    multiplied in during the PSUM->SBUF copy (bf16 attnT [hd=128, si]);
    V-bias added post-normalize (P@(V + 1*vb) = PV + denom*vb).
  - out_proj: partial = attnT.T @ woT slice -> [tok, E] bf16 partial output;
    host sums the 8 partials in fp32 and adds out_proj_bias.

Scheduling: emission order is a chunk-level software pipeline. Projection
work is chopped into ~2-4us "morsels" on a queue; the attention sj-loop pops
one morsel BEFORE each sj iteration (so producers precede consumers in PE
program order — the queue is deadline-ordered), keeping ScalarE (exp) fed
while the PE fills its slack with projection work. finish(i-1) (normalize +
out_proj) is emitted after compute(i) so the PE never idles long enough to
re-throttle (HAM).

attn_mask is all-zeros in this problem's setup_inputs; a masked variant of
the graph (maskT added to scores pre-exp) is built only if a nonzero mask
ever shows up.
"""

import numpy as np
import ml_dtypes
from collections import deque
from contextlib import ExitStack

import concourse.bass as bass
import concourse.bacc as bacc
import concourse.tile as tile
import concourse.mybir as mybir
from concourse.bass_utils import run_bass_kernel_spmd
from concourse.bass import ts, ds
from concourse.masks import make_identity

BF16 = mybir.dt.bfloat16
F32 = mybir.dt.float32

P = 128
E = 1024
H = 16
D = 64
B = 2
S = 2048
T = B * S            # 4096 tokens
ET = E // P          # 8 e-tiles
TCH = 512            # projection token chunk
NTCH = T // TCH      # 8
SC = 512             # attention si chunk
NSC = S // SC        # 4 per batch
SJT = S // P         # 16 sj tiles per batch
NCORES = 8
SCALE = float(D) ** -0.5

_nc_cache = {}


def _build_nc(use_mask: bool):
    nc = bacc.Bacc("TRN2", target_bir_lowering=False, debug=False,
                   num_devices=NCORES)
    xT_d = nc.dram_tensor("xT", [P, NTCH, ET, TCH], BF16,
                          kind="ExternalInput").ap()
    wq_d = nc.dram_tensor("wqT", [P, ET, P], BF16, kind="ExternalInput").ap()
    wk_d = nc.dram_tensor("wkT", [P, ET, P], BF16, kind="ExternalInput").ap()
    wv_d = nc.dram_tensor("wvT", [P, ET, P], BF16, kind="ExternalInput").ap()
    qb_d = nc.dram_tensor("qb", [P, 1], F32, kind="ExternalInput").ap()
    kb_d = nc.dram_tensor("kb", [P, 1], F32, kind="ExternalInput").ap()
    vb_d = nc.dram_tensor("vb", [P, 1], F32, kind="ExternalInput").ap()
    wo_d = nc.dram_tensor("woT", [P, E], BF16, kind="ExternalInput").ap()
    mask_d = None
    if use_mask:
        mask_d = nc.dram_tensor("maskT", [B, S, S], BF16,
                                kind="ExternalInput").ap()
    out_d = nc.dram_tensor("out", [T, E], BF16, kind="ExternalOutput").ap()

    with tile.TileContext(nc) as tc, ExitStack() as ctx:
        persist = ctx.enter_context(tc.tile_pool(name="persist", bufs=1))
        work = ctx.enter_context(tc.tile_pool(name="work", bufs=2))
        expp = ctx.enter_context(tc.tile_pool(name="expp", bufs=6))
        psum = ctx.enter_context(tc.tile_pool(name="psum", bufs=2, space="PSUM"))

        # ---- persistent SBUF tensors ----
        xT = persist.tile([P, NTCH, ET, TCH], BF16, name="xT_sb", tag="xT_sb")
        wq = persist.tile([P, ET, P], BF16, name="wq_sb", tag="wq_sb")
        wk = persist.tile([P, ET, P], BF16, name="wk_sb", tag="wk_sb")
        wv = persist.tile([P, ET, P], BF16, name="wv_sb", tag="wv_sb")
        qb = persist.tile([P, 1], F32, name="qb_sb", tag="qb_sb")
        kb = persist.tile([P, 1], F32, name="kb_sb", tag="kb_sb")
        vb = persist.tile([P, 1], F32, name="vb_sb", tag="vb_sb")
        wo = persist.tile([P, E], BF16, name="wo_sb", tag="wo_sb")
        ident = persist.tile([P, P], BF16, name="ident_sb", tag="ident_sb")
        QT = persist.tile([P, T], BF16, name="QT_sb", tag="QT_sb")
        KT = persist.tile([P, T], BF16, name="KT_sb", tag="KT_sb")
        V = persist.tile([P, T // P, 2 * (D + 1)], BF16, name="V_sb", tag="V_sb")

        nc.sync.dma_start(wq[:], wq_d)
        nc.sync.dma_start(wk[:], wk_d)
        nc.sync.dma_start(wv[:], wv_d)
        nc.sync.dma_start(qb[:], qb_d)
        nc.sync.dma_start(kb[:], kb_d)
        nc.sync.dma_start(vb[:], vb_d)
        nc.sync.dma_start(wo[:], wo_d)
        for t in range(NTCH):
            nc.sync.dma_start(xT[:, t], xT_d[:, t])

        make_identity(nc, ident[:])
        # ones columns for the softmax-denominator augmentation of V
        nc.vector.memset(V[:, :, D:D + 1], 1.0)
        nc.vector.memset(V[:, :, 2 * D + 1:2 * D + 2], 1.0)

        # ---- stage A morsels ----
        def proj_qk(t, is_q):
            w, bcol, dst = (wq, qb, QT) if is_q else (wk, kb, KT)
            nm = "q" if is_q else "k"
            ps = psum.tile([P, TCH], F32, name=f"{nm}_ps_{t}", tag="a")
            for e in range(ET):
                nc.tensor.matmul(ps[:], w[:, e, :], xT[:, t, e, :],
                                 start=(e == 0), stop=(e == ET - 1))
            nc.vector.tensor_scalar_add(dst[:, ts(t, TCH)], ps[:], bcol[:])

        vt_tiles = {}

        def proj_v_mm(t):
            ps = psum.tile([P, TCH], F32, name=f"vt_ps_{t}", tag="a")
            for e in range(ET):
                nc.tensor.matmul(ps[:], wv[:, e, :], xT[:, t, e, :],
                                 start=(e == 0), stop=(e == ET - 1))
            vt_sb = work.tile([P, TCH], BF16, name=f"vt_sb_{t}", tag="vt",
                              bufs=2)
            nc.vector.tensor_copy(out=vt_sb[:], in_=ps[:])
            vt_tiles[t] = vt_sb

        def proj_v_tr(t, s4s):
            vt_sb = vt_tiles[t]
            for s4 in s4s:
                jt = t * (TCH // P) + s4
                pt = psum.tile([P, P], BF16, name=f"vtr_ps_{jt}", tag="a")
                nc.tensor.transpose(pt[:], vt_sb[:, ds(s4 * P, P)], ident[:])
                nc.vector.tensor_copy(
                    out=V[:, jt].rearrange("p (g c) -> p g c", g=2)[:, :, 0:D],
                    in_=pt.rearrange("p (g c) -> p g c", g=2))

        # ---- stage B: attention sj-loop; pops one morsel per sj tile ----
        work_q = deque()

        def attn_compute(b, sci, npop=1):
            si0 = b * S + sci * SC
            pvA = psum.tile([D + 1, SC], F32, name=f"pvA_{b}_{sci}", tag="pv")
            pvB = psum.tile([D + 1, SC], F32, name=f"pvB_{b}_{sci}", tag="pv")

            def pv_mms(sjt):
                jt = b * SJT + sjt
                expab = exp_tiles[sjt]
                nc.tensor.matmul(pvA[:], V[:, jt, 0:D + 1], expab[:, 0:SC],
                                 start=(sjt == 0), stop=(sjt == SJT - 1))
                nc.tensor.matmul(pvB[:], V[:, jt, D + 1:2 * (D + 1)],
                                 expab[:, SC:2 * SC],
                                 start=(sjt == 0), stop=(sjt == SJT - 1))

            exp_tiles = {}
            for sjt in range(SJT):
                # emit queued projection morsels BEFORE this iteration so
                # their tiles are written earlier in PE program order than
                # the scores/PV matmuls that read them (deadline-ordered)
                for _ in range(npop):
                    if work_q:
                        work_q.popleft()()
                # PV trails by two sj tiles, emitted before this iteration's
                # scores: exp(sjt-2) is already complete (it gates the scores
                # PSUM slot), so the PV wait is pre-cleared and every
                # LDWEIGHTS in the stream pipelines behind a running matmul
                if sjt >= 2:
                    pv_mms(sjt - 2)
                jt = b * SJT + sjt
                scs = psum.tile([P, 2 * SC], F32, name=f"scs_{b}_{sci}_{sjt}",
                                tag="sc")
                nc.tensor.matmul(scs[:, 0:SC], KT[0:D, ds(jt * P, P)],
                                 QT[0:D, ds(si0, SC)], start=True, stop=True,
                                 tile_position=(0, 0))
                nc.tensor.matmul(scs[:, SC:2 * SC], KT[D:P, ds(jt * P, P)],
                                 QT[D:P, ds(si0, SC)], start=True, stop=True,
                                 tile_position=(64, 0))
                if use_mask:
                    mt = work.tile([P, SC], BF16, name=f"mt_{b}_{sci}_{sjt}",
                                   tag="mask", bufs=3)
                    nc.sync.dma_start(
                        mt[:], mask_d[b, ds(sjt * P, P), ds(sci * SC, SC)])
                    nc.vector.tensor_tensor(
                        out=scs.rearrange("p (g c) -> p g c", g=2),
                        in0=scs.rearrange("p (g c) -> p g c", g=2),
                        in1=mt[:, None, :].to_broadcast([P, 2, SC]),
                        op=mybir.AluOpType.add)
                expab = expp.tile([P, 2 * SC], BF16, name=f"ex_{b}_{sci}_{sjt}",
                                  tag="exp")
                nc.scalar.activation(expab[:], scs[:],
                                     mybir.ActivationFunctionType.Exp)
                exp_tiles[sjt] = expab
            pv_mms(SJT - 2)
            pv_mms(SJT - 1)
            return pvA, pvB

        # ---- normalize + out_proj for a finished (b, si-chunk) ----
        def out_proj(b, sci, attnT, tts):
            si0 = b * S + sci * SC
            for tt in tts:
                tok0 = si0 + tt * P
                ops = psum.tile([P, E], F32, name=f"o_ps_{b}_{sci}_{tt}",
                                tag="sc")
                for ne in range(E // 512):
                    nc.tensor.matmul(ops[:, ts(ne, 512)], attnT[:, ts(tt, P)],
                                     wo[:, ts(ne, 512)], start=True, stop=True)
                outt = work.tile([P, E], BF16, name=f"outt_{b}_{sci}_{tt}",
                                 tag="outt", bufs=3)
                nc.vector.tensor_copy(out=outt[:], in_=ops[:])
                nc.sync.dma_start(out_d[ds(tok0, P), :], outt[:])

        def attn_finish(b, sci, pvA, pvB):
            attnT = work.tile([P, SC], BF16, name=f"attnT_{b}_{sci}",
                              tag="attnT", bufs=3)
            for hh, pv in ((0, pvA), (1, pvB)):
                den = work.tile([1, SC], F32, name=f"den_{b}_{sci}_{hh}",
                                tag="den", bufs=4)
                nc.vector.tensor_copy(out=den[:], in_=pv[D:D + 1, :])
                rec = work.tile([1, SC], F32, name=f"rec_{b}_{sci}_{hh}",
                                tag="rec", bufs=4)
                nc.vector.reciprocal_approx_fast(out=rec[:], in_=den[:])
                bc = work.tile([D, SC], F32, name=f"bc_{b}_{sci}_{hh}",
                               tag="bc", bufs=4)
                nc.gpsimd.partition_broadcast(bc[:], rec[:])
                nc.vector.tensor_tensor(out=attnT[hh * D:(hh + 1) * D, :],
                                        in0=pv[0:D, :], in1=bc[:],
                                        op=mybir.AluOpType.mult)
                # V-bias: P@(V + 1*vb) = PV + denom*vb -> add vb post-normalize
                nc.vector.tensor_scalar_add(attnT[hh * D:(hh + 1) * D, :],
                                            attnT[hh * D:(hh + 1) * D, :],
                                            vb[hh * D:(hh + 1) * D, :])
            # out_proj goes on the morsel queue (popped during the next
            # chunk's sj-loop) so it doesn't block the PE at chunk boundary
            work_q.append(lambda: out_proj(b, sci, attnT, (0, 1)))
            work_q.append(lambda: out_proj(b, sci, attnT, (2, 3)))

        # ---- emission ----
        # prologue: minimal JIT set for attention chunk (0,0)
        proj_qk(0, False)
        proj_v_mm(0)
        proj_v_tr(0, (0, 1))
        proj_v_tr(0, (2, 3))
        proj_qk(0, True)
        # morsel queue in deadline order (1 pop/sjt meets every deadline:
        # kv chunk c lands at pops 4c-3..4c, needed at sj tile 4c)
        for t in range(1, 4):
            work_q.append(lambda t=t: proj_qk(t, False))
            work_q.append(lambda t=t: proj_v_mm(t))
            work_q.append(lambda t=t: proj_v_tr(t, (0, 1)))
            work_q.append(lambda t=t: proj_v_tr(t, (2, 3)))
        work_q.append(lambda: proj_qk(1, True))
        for t in range(4, NTCH):
            work_q.append(lambda t=t: proj_qk(t, False))
            work_q.append(lambda t=t: proj_v_mm(t))
            work_q.append(lambda t=t: proj_v_tr(t, (0, 1)))
            work_q.append(lambda t=t: proj_v_tr(t, (2, 3)))
            work_q.append(lambda t=t: proj_qk(t - 2, True))
        work_q.append(lambda: proj_qk(6, True))
        work_q.append(lambda: proj_qk(7, True))

        chunks = [(0, s) for s in range(NSC)] + [(1, s) for s in range(NSC)]
        prev = None
        for b, sci in chunks:
            pv = attn_compute(b, sci)
            if prev is not None:
                attn_finish(*prev)
            prev = (b, sci, *pv)
        attn_finish(*prev)
        while work_q:
            work_q.popleft()()

    nc.compile()
    return nc


def _get_nc(use_mask: bool):
    if use_mask not in _nc_cache:
        _nc_cache[use_mask] = _build_nc(use_mask)
    return _nc_cache[use_mask]


def _prep_in_maps(x, attn_mask, in_proj_weight, in_proj_bias, out_proj_weight,
                  lora_a, lora_b, use_mask):
    bf = ml_dtypes.bfloat16

    def wtile(w2d):  # [E, M] -> [P, ET, M] contiguous
        m = w2d.shape[1]
        return np.ascontiguousarray(
            w2d.reshape(ET, P, m).transpose(1, 0, 2)).astype(bf)

    xf = x.reshape(T, E)
    xT = np.ascontiguousarray(
        xf.reshape(NTCH, TCH, ET, P).transpose(3, 0, 2, 1)).astype(bf)
    # fold the (linear) LoRA delta into the projection weights
    w_eff = in_proj_weight + lora_b @ lora_a
    maskT = None
    if use_mask:
        maskT = np.ascontiguousarray(attn_mask.transpose(0, 2, 1)).astype(bf)
    in_maps = []
    for c in range(NCORES):
        h0 = 2 * c
        qs = slice(h0 * D, (h0 + 2) * D)
        ks = slice(E + h0 * D, E + (h0 + 2) * D)
        vs = slice(2 * E + h0 * D, 2 * E + (h0 + 2) * D)
        m = {
            "xT": xT,
            "wqT": wtile(w_eff[qs, :].T * SCALE),
            "wkT": wtile(w_eff[ks, :].T),
            "wvT": wtile(w_eff[vs, :].T),
            "qb": np.ascontiguousarray((in_proj_bias[qs] * SCALE)[:, None]).astype(np.float32),
            "kb": np.ascontiguousarray(in_proj_bias[ks][:, None]).astype(np.float32),
            "vb": np.ascontiguousarray(in_proj_bias[vs][:, None]).astype(np.float32),
            "woT": np.ascontiguousarray(out_proj_weight[:, h0 * D:(h0 + 2) * D].T).astype(bf),
        }
        if use_mask:
            m["maskT"] = maskT
        in_maps.append(m)
    return in_maps


def kernel(x, attn_mask, in_proj_weight, in_proj_bias, out_proj_weight,
           out_proj_bias, lora_a, lora_b, _trace=False):
    x = np.asarray(x, dtype=np.float32)
    attn_mask = np.asarray(attn_mask, dtype=np.float32)
    in_proj_weight = np.asarray(in_proj_weight, dtype=np.float32)
    in_proj_bias = np.asarray(in_proj_bias, dtype=np.float32)
    out_proj_weight = np.asarray(out_proj_weight, dtype=np.float32)
    out_proj_bias = np.asarray(out_proj_bias, dtype=np.float32)
    lora_a = np.asarray(lora_a, dtype=np.float32)
    lora_b = np.asarray(lora_b, dtype=np.float32)

    use_mask = bool(np.any(attn_mask))
    nc = _get_nc(use_mask)
    in_maps = _prep_in_maps(x, attn_mask, in_proj_weight, in_proj_bias,
                            out_proj_weight, lora_a, lora_b, use_mask)
    res = run_bass_kernel_spmd(nc, in_maps, core_ids=list(range(NCORES)),
                               trace=_trace)
    acc = np.zeros((T, E), np.float32)
    for c in range(NCORES):
        acc += res.results[c]["out"].astype(np.float32)
    acc += out_proj_bias[None, :]
    out = acc.reshape(B, S, E)
    if _trace:
        kernel._last_exec_time_ns = res.exec_time_ns
        kernel._last_trace = (res.instructions_and_trace[1]
                              if res.instructions_and_trace else None)
    return out


# revision 34
# speedup vs baseline: 1.4523x; 1.0140x over previous
"""Trainium2 Bass kernel for nn_Attention_29807073034381.

Multi-head attention (B=2, S=2048, E=1024, H=16, D=64) with LoRA-augmented QKV
projection, sharded 2-heads-per-core across 8 NeuronCores (tensor parallel).

Key choices:
  - LoRA is linear, so the host folds it into the projection weights
    (W_eff = W + lora_b @ lora_a) and slices/transposes per core; the softmax
    scaling is folded into Wq/bias_q. No LoRA compute on device.
  - All device compute in bf16 with fp32 PSUM accumulation (rel-err gate 2e-2;
    measured ~6e-3). Host passes x pre-transposed/pre-tiled so every matmul
    contraction sits on the SBUF partition axis and every DMA line is 8KB.
  - projection: QT/KT [128, T] (head-dim on partitions, weight-stationary);
    V computed as VT then PE-transposed into [tok, dv] tiles augmented with a
    ones column; Q/K bias added as a per-partition scalar at PSUM-copy time.
  - attention per (b, si-chunk of 512): scoresT tiles [sj=128, si=512] for
    both heads packed into one [128, 1024] PSUM tile via K=64 row-packed
    matmuls (tile_position (0,0)/(64,0)); one ACT exp per sj-tile
    (PSUM f32 -> SBUF bf16); PV matmuls against ones-augmented V accumulate
    attnoutT and the softmax denominator in one PSUM group.
  - normalize via reciprocal_approx_fast + gpsimd partition_broadcast,
    multiplied in during the PSUM->SBUF copy (bf16 attnT [hd=128, si]);
    V-bias added post-normalize (P@(V + 1*vb) = PV + denom*vb).
  - out_proj: partial = attnT.T @ woT slice -> [tok, E] bf16 partial output;
    host sums the 8 partials in fp32 and adds out_proj_bias.

Scheduling: emission order is a chunk-level software pipeline. Projection
work is chopped into ~2-4us "morsels" on a queue; the attention sj-loop pops
one morsel BEFORE each sj iteration (so producers precede consumers in PE
program order — the queue is deadline-ordered), keeping ScalarE (exp) fed
while the PE fills its slack with projection work. finish(i-1) (normalize +
out_proj) is emitted after compute(i) so the PE never idles long enough to
re-throttle (HAM).

attn_mask is all-zeros in this problem's setup_inputs; a masked variant of
the graph (maskT added to scores pre-exp) is built only if a nonzero mask
ever shows up.
"""

import numpy as np
import ml_dtypes
from collections import deque
from contextlib import ExitStack

import concourse.bass as bass
import concourse.bacc as bacc
import concourse.tile as tile
import concourse.mybir as mybir
from concourse.bass_utils import run_bass_kernel_spmd
from concourse.bass import ts, ds
from concourse.masks import make_identity

BF16 = mybir.dt.bfloat16
F32 = mybir.dt.float32

P = 128
E = 1024
H = 16
D = 64
B = 2
S = 2048
T = B * S            # 4096 tokens
ET = E // P          # 8 e-tiles
TCH = 512            # projection token chunk
NTCH = T // TCH      # 8
SC = 512             # attention si chunk
NSC = S // SC        # 4 per batch
SJT = S // P         # 16 sj tiles per batch
NCORES = 8
SCALE = float(D) ** -0.5

_nc_cache = {}


def _build_nc(use_mask: bool):
    nc = bacc.Bacc("TRN2", target_bir_lowering=False, debug=False,
                   num_devices=NCORES)
    xT_d = nc.dram_tensor("xT", [P, NTCH, ET, TCH], BF16,
                          kind="ExternalInput").ap()
    wq_d = nc.dram_tensor("wqT", [P, ET, P], BF16, kind="ExternalInput").ap()
    wk_d = nc.dram_tensor("wkT", [P, ET, P], BF16, kind="ExternalInput").ap()
    wv_d = nc.dram_tensor("wvT", [P, ET, P], BF16, kind="ExternalInput").ap()
    qb_d = nc.dram_tensor("qb", [P, 1], F32, kind="ExternalInput").ap()
    kb_d = nc.dram_tensor("kb", [P, 1], F32, kind="ExternalInput").ap()
    vb_d = nc.dram_tensor("vb", [P, 1], F32, kind="ExternalInput").ap()
    wo_d = nc.dram_tensor("woT", [P, E], BF16, kind="ExternalInput").ap()
    mask_d = None
    if use_mask:
        mask_d = nc.dram_tensor("maskT", [B, S, S], BF16,
                                kind="ExternalInput").ap()
    out_d = nc.dram_tensor("out", [T, E], BF16, kind="ExternalOutput").ap()

    with tile.TileContext(nc) as tc, ExitStack() as ctx:
        persist = ctx.enter_context(tc.tile_pool(name="persist", bufs=1))
        work = ctx.enter_context(tc.tile_pool(name="work", bufs=2))
        expp = ctx.enter_context(tc.tile_pool(name="expp", bufs=6))
        psum = ctx.enter_context(tc.tile_pool(name="psum", bufs=2, space="PSUM"))

        # ---- persistent SBUF tensors ----
        xT = persist.tile([P, NTCH, ET, TCH], BF16, name="xT_sb", tag="xT_sb")
        wq = persist.tile([P, ET, P], BF16, name="wq_sb", tag="wq_sb")
        wk = persist.tile([P, ET, P], BF16, name="wk_sb", tag="wk_sb")
        wv = persist.tile([P, ET, P], BF16, name="wv_sb", tag="wv_sb")
        qb = persist.tile([P, 1], F32, name="qb_sb", tag="qb_sb")
        kb = persist.tile([P, 1], F32, name="kb_sb", tag="kb_sb")
        vb = persist.tile([P, 1], F32, name="vb_sb", tag="vb_sb")
        wo = persist.tile([P, E], BF16, name="wo_sb", tag="wo_sb")
        ident = persist.tile([P, P], BF16, name="ident_sb", tag="ident_sb")
        QT = persist.tile([P, T], BF16, name="QT_sb", tag="QT_sb")
        KT = persist.tile([P, T], BF16, name="KT_sb", tag="KT_sb")
        V = persist.tile([P, T // P, 2 * (D + 1)], BF16, name="V_sb", tag="V_sb")

        nc.sync.dma_start(wq[:], wq_d)
        nc.sync.dma_start(wk[:], wk_d)
        nc.sync.dma_start(wv[:], wv_d)
        nc.sync.dma_start(qb[:], qb_d)
        nc.sync.dma_start(kb[:], kb_d)
        nc.sync.dma_start(vb[:], vb_d)
        nc.sync.dma_start(wo[:], wo_d)
        for t in range(NTCH):
            nc.sync.dma_start(xT[:, t], xT_d[:, t])

        make_identity(nc, ident[:])
        # ones columns for the softmax-denominator augmentation of V
        nc.vector.memset(V[:, :, D:D + 1], 1.0)
        nc.vector.memset(V[:, :, 2 * D + 1:2 * D + 2], 1.0)

        # ---- stage A morsels ----
        def proj_qk(t, is_q):
            w, bcol, dst = (wq, qb, QT) if is_q else (wk, kb, KT)
            nm = "q" if is_q else "k"
            ps = psum.tile([P, TCH], F32, name=f"{nm}_ps_{t}", tag="a")
            for e in range(ET):
                nc.tensor.matmul(ps[:], w[:, e, :], xT[:, t, e, :],
                                 start=(e == 0), stop=(e == ET - 1))
            nc.vector.tensor_scalar_add(dst[:, ts(t, TCH)], ps[:], bcol[:])

        vt_tiles = {}

        def proj_v_mm(t):
            ps = psum.tile([P, TCH], F32, name=f"vt_ps_{t}", tag="a")
            for e in range(ET):
                nc.tensor.matmul(ps[:], wv[:, e, :], xT[:, t, e, :],
                                 start=(e == 0), stop=(e == ET - 1))
            vt_sb = work.tile([P, TCH], BF16, name=f"vt_sb_{t}", tag="vt",
                              bufs=2)
            nc.vector.tensor_copy(out=vt_sb[:], in_=ps[:])
            vt_tiles[t] = vt_sb

        def proj_v_tr(t, s4s):
            vt_sb = vt_tiles[t]
            for s4 in s4s:
                jt = t * (TCH // P) + s4
                pt = psum.tile([P, P], BF16, name=f"vtr_ps_{jt}", tag="a")
                nc.tensor.transpose(pt[:], vt_sb[:, ds(s4 * P, P)], ident[:])
                nc.vector.tensor_copy(
                    out=V[:, jt].rearrange("p (g c) -> p g c", g=2)[:, :, 0:D],
                    in_=pt.rearrange("p (g c) -> p g c", g=2))

        # ---- stage B: attention sj-loop; pops one morsel per sj tile ----
        work_q = deque()

        def attn_compute(b, sci, pace=2):
            si0 = b * S + sci * SC
            pvA = psum.tile([D + 1, SC], F32, name=f"pvA_{b}_{sci}", tag="pv")
            pvB = psum.tile([D + 1, SC], F32, name=f"pvB_{b}_{sci}", tag="pv")

            def pv_mms(sjt):
                jt = b * SJT + sjt
                expab = exp_tiles[sjt]
                nc.tensor.matmul(pvA[:], V[:, jt, 0:D + 1], expab[:, 0:SC],
                                 start=(sjt == 0), stop=(sjt == SJT - 1))
                nc.tensor.matmul(pvB[:], V[:, jt, D + 1:2 * (D + 1)],
                                 expab[:, SC:2 * SC],
                                 start=(sjt == 0), stop=(sjt == SJT - 1))

            exp_tiles = {}
            for sjt in range(SJT):
                # emit queued projection morsels BEFORE this iteration so
                # their tiles are written earlier in PE program order than
                # the scores/PV matmuls that read them (deadline-ordered).
                # pace>1 spreads the queue across the whole kernel so the PE
                # always has dense work (keeps HAM at full clock).
                if sjt % pace == 0 and work_q:
                    work_q.popleft()()
                # PV trails by two sj tiles, emitted before this iteration's
                # scores: exp(sjt-2) is already complete (it gates the scores
                # PSUM slot), so the PV wait is pre-cleared and every
                # LDWEIGHTS in the stream pipelines behind a running matmul
                if sjt >= 2:
                    pv_mms(sjt - 2)
                jt = b * SJT + sjt
                scs = psum.tile([P, 2 * SC], F32, name=f"scs_{b}_{sci}_{sjt}",
                                tag="sc")
                nc.tensor.matmul(scs[:, 0:SC], KT[0:D, ds(jt * P, P)],
                                 QT[0:D, ds(si0, SC)], start=True, stop=True,
                                 tile_position=(0, 0))
                nc.tensor.matmul(scs[:, SC:2 * SC], KT[D:P, ds(jt * P, P)],
                                 QT[D:P, ds(si0, SC)], start=True, stop=True,
                                 tile_position=(64, 0))
                if use_mask:
                    mt = work.tile([P, SC], BF16, name=f"mt_{b}_{sci}_{sjt}",
                                   tag="mask", bufs=3)
                    nc.sync.dma_start(
                        mt[:], mask_d[b, ds(sjt * P, P), ds(sci * SC, SC)])
                    nc.vector.tensor_tensor(
                        out=scs.rearrange("p (g c) -> p g c", g=2),
                        in0=scs.rearrange("p (g c) -> p g c", g=2),
                        in1=mt[:, None, :].to_broadcast([P, 2, SC]),
                        op=mybir.AluOpType.add)
                expab = expp.tile([P, 2 * SC], BF16, name=f"ex_{b}_{sci}_{sjt}",
                                  tag="exp")
                nc.scalar.activation(expab[:], scs[:],
                                     mybir.ActivationFunctionType.Exp)
                exp_tiles[sjt] = expab
            pv_mms(SJT - 2)
            pv_mms(SJT - 1)
            return pvA, pvB

        # ---- normalize + out_proj for a finished (b, si-chunk) ----
        def out_proj(b, sci, attnT, tts):
            si0 = b * S + sci * SC
            for tt in tts:
                tok0 = si0 + tt * P
                ops = psum.tile([P, E], F32, name=f"o_ps_{b}_{sci}_{tt}",
                                tag="sc")
                for ne in range(E // 512):
                    nc.tensor.matmul(ops[:, ts(ne, 512)], attnT[:, ts(tt, P)],
                                     wo[:, ts(ne, 512)], start=True, stop=True)
                outt = work.tile([P, E], BF16, name=f"outt_{b}_{sci}_{tt}",
                                 tag="outt", bufs=3)
                nc.vector.tensor_copy(out=outt[:], in_=ops[:])
                nc.sync.dma_start(out_d[ds(tok0, P), :], outt[:])

        def attn_finish(b, sci, pvA, pvB):
            attnT = work.tile([P, SC], BF16, name=f"attnT_{b}_{sci}",
                              tag="attnT", bufs=3)
            for hh, pv in ((0, pvA), (1, pvB)):
                den = work.tile([1, SC], F32, name=f"den_{b}_{sci}_{hh}",
                                tag="den", bufs=4)
                nc.vector.tensor_copy(out=den[:], in_=pv[D:D + 1, :])
                rec = work.tile([1, SC], F32, name=f"rec_{b}_{sci}_{hh}",
                                tag="rec", bufs=4)
                nc.vector.reciprocal_approx_fast(out=rec[:], in_=den[:])
                bc = work.tile([D, SC], F32, name=f"bc_{b}_{sci}_{hh}",
                               tag="bc", bufs=4)
                nc.gpsimd.partition_broadcast(bc[:], rec[:])
                nc.vector.tensor_tensor(out=attnT[hh * D:(hh + 1) * D, :],
                                        in0=pv[0:D, :], in1=bc[:],
                                        op=mybir.AluOpType.mult)
                # V-bias: P@(V + 1*vb) = PV + denom*vb -> add vb post-normalize
                nc.vector.tensor_scalar_add(attnT[hh * D:(hh + 1) * D, :],
                                            attnT[hh * D:(hh + 1) * D, :],
                                            vb[hh * D:(hh + 1) * D, :])
            # out_proj goes on the morsel queue (popped during the next
            # chunk's sj-loop) so it doesn't block the PE at chunk boundary
            work_q.append(lambda: out_proj(b, sci, attnT, (0, 1)))
            work_q.append(lambda: out_proj(b, sci, attnT, (2, 3)))

        # ---- emission ----
        # prologue: minimal JIT set for attention chunk (0,0)
        proj_qk(0, False)
        proj_v_mm(0)
        proj_v_tr(0, (0, 1))
        proj_v_tr(0, (2, 3))
        proj_qk(0, True)
        # morsel queue in deadline order (1 pop/sjt meets every deadline:
        # kv chunk c lands at pops 4c-3..4c, needed at sj tile 4c)
        for t in range(1, 4):
            work_q.append(lambda t=t: proj_qk(t, False))
            work_q.append(lambda t=t: proj_v_mm(t))
            work_q.append(lambda t=t: proj_v_tr(t, (0, 1)))
            work_q.append(lambda t=t: proj_v_tr(t, (2, 3)))
        work_q.append(lambda: proj_qk(1, True))
        for t in range(4, NTCH):
            work_q.append(lambda t=t: proj_qk(t, False))
            work_q.append(lambda t=t: proj_v_mm(t))
            work_q.append(lambda t=t: proj_v_tr(t, (0, 1)))
            work_q.append(lambda t=t: proj_v_tr(t, (2, 3)))
            work_q.append(lambda t=t: proj_qk(t - 2, True))
        work_q.append(lambda: proj_qk(6, True))
        work_q.append(lambda: proj_qk(7, True))

        chunks = [(0, s) for s in range(NSC)] + [(1, s) for s in range(NSC)]
        prev = None
        for i, (b, sci) in enumerate(chunks):
            pv = attn_compute(b, sci, pace=1 if i == 0 else 2)
            if prev is not None:
                attn_finish(*prev)
            prev = (b, sci, *pv)
        attn_finish(*prev)
        while work_q:
            work_q.popleft()()

    nc.compile()
    return nc


def _get_nc(use_mask: bool):
    if use_mask not in _nc_cache:
        _nc_cache[use_mask] = _build_nc(use_mask)
    return _nc_cache[use_mask]


def _prep_in_maps(x, attn_mask, in_proj_weight, in_proj_bias, out_proj_weight,
                  lora_a, lora_b, use_mask):
    bf = ml_dtypes.bfloat16

    def wtile(w2d):  # [E, M] -> [P, ET, M] contiguous
        m = w2d.shape[1]
        return np.ascontiguousarray(
            w2d.reshape(ET, P, m).transpose(1, 0, 2)).astype(bf)

    xf = x.reshape(T, E)
    xT = np.ascontiguousarray(
        xf.reshape(NTCH, TCH, ET, P).transpose(3, 0, 2, 1)).astype(bf)
    # fold the (linear) LoRA delta into the projection weights
    w_eff = in_proj_weight + lora_b @ lora_a
    maskT = None
    if use_mask:
        maskT = np.ascontiguousarray(attn_mask.transpose(0, 2, 1)).astype(bf)
    in_maps = []
    for c in range(NCORES):
        h0 = 2 * c
        qs = slice(h0 * D, (h0 + 2) * D)
        ks = slice(E + h0 * D, E + (h0 + 2) * D)
        vs = slice(2 * E + h0 * D, 2 * E + (h0 + 2) * D)
        m = {
            "xT": xT,
            "wqT": wtile(w_eff[qs, :].T * SCALE),
            "wkT": wtile(w_eff[ks, :].T),
            "wvT": wtile(w_eff[vs, :].T),
            "qb": np.ascontiguousarray((in_proj_bias[qs] * SCALE)[:, None]).astype(np.float32),
            "kb": np.ascontiguousarray(in_proj_bias[ks][:, None]).astype(np.float32),
            "vb": np.ascontiguousarray(in_proj_bias[vs][:, None]).astype(np.float32),
            "woT": np.ascontiguousarray(out_proj_weight[:, h0 * D:(h0 + 2) * D].T).astype(bf),
        }
        if use_mask:
            m["maskT"] = maskT
        in_maps.append(m)
    return in_maps


def kernel(x, attn_mask, in_proj_weight, in_proj_bias, out_proj_weight,
           out_proj_bias, lora_a, lora_b, _trace=False):
    x = np.asarray(x, dtype=np.float32)
    attn_mask = np.asarray(attn_mask, dtype=np.float32)
    in_proj_weight = np.asarray(in_proj_weight, dtype=np.float32)
    in_proj_bias = np.asarray(in_proj_bias, dtype=np.float32)
    out_proj_weight = np.asarray(out_proj_weight, dtype=np.float32)
    out_proj_bias = np.asarray(out_proj_bias, dtype=np.float32)
    lora_a = np.asarray(lora_a, dtype=np.float32)
    lora_b = np.asarray(lora_b, dtype=np.float32)

    use_mask = bool(np.any(attn_mask))
    nc = _get_nc(use_mask)
    in_maps = _prep_in_maps(x, attn_mask, in_proj_weight, in_proj_bias,
                            out_proj_weight, lora_a, lora_b, use_mask)
    res = run_bass_kernel_spmd(nc, in_maps, core_ids=list(range(NCORES)),
                               trace=_trace)
    acc = np.zeros((T, E), np.float32)
    for c in range(NCORES):
        acc += res.results[c]["out"].astype(np.float32)
    acc += out_proj_bias[None, :]
    out = acc.reshape(B, S, E)
    if _trace:
        kernel._last_exec_time_ns = res.exec_time_ns
        kernel._last_trace = (res.instructions_and_trace[1]
                              if res.instructions_and_trace else None)
    return out
